# revision 45
# baseline (speedup 1.0000x reference)
"""Trainium2 Bass kernel for LocalGlobalEnvEncoder (GCN + MHA fusion).

Sharding: nodes are split across the 8 cores (1024 dest nodes / queries each).
 - GCN: edges bucketed by destination node-tile on host (layout only); source
   features are laid out in edge order on host (a pure gather / relayout), so
   the device streams them with plain sequential DMA instead of per-row
   indirect gathers. Per-edge 1/sqrt(d_row) weights are folded into the
   one-hot scatter matrix (built on DVE with a fused is_equal*mult chain) and
   the scatter-add runs on the PE in bf16.  Degrees are integer counts
   (host-side bincount relayout); all floating-point math (rsqrt, scaling,
   matmuls) happens on device.
 - MHA: query-sharded attention, K/V computed redundantly per core in bf16.
   Scores are kept transposed ([key, query]); exp runs on the ACT engine
   writing bf16; attn@V uses V as the stationary operand accumulating
   O^T [head_dim+1, queries] over key tiles, with softmax denominators coming
   from an appended ones-column in V.  Normalization (1/den) is applied
   column-wise via a gpsimd partition-broadcast.
All floating-point math happens on device; the host only re-lays-out inputs.
"""
import sys
sys.path.insert(0, '/opt/trn_rl_repo')
import numpy as np
import ml_dtypes
import concourse.bass as bass
import concourse.tile as tile
from concourse import bacc, mybir
from concourse.bass_utils import run_bass_kernel_spmd

F32 = mybir.dt.float32
BF16 = mybir.dt.bfloat16
I32 = mybir.dt.int32
AF = mybir.ActivationFunctionType
OP = mybir.AluOpType
AX = mybir.AxisListType

N, E, C, OUTC, H, DH = 8192, 262144, 256, 256, 4, 64
NCORES = 8
NPC = N // NCORES          # nodes per core = 1024
P = 128
NT_LOC = NPC // P          # node tiles per core = 8
NT_GLOB = N // P           # global node tiles = 64
EXP_BIAS = -12.0           # uniform shift inside softmax exp; cancels in the ratio

LAST_RESULTS = None        # stashed BassKernelResults for test harness introspection


def _build(TPT):
    """Build the single SPMD Bass program. TPT = edge tiles per node-tile segment."""
    nc = bacc.Bacc('TRN2', target_bir_lowering=False, debug=False, num_devices=NCORES)
    TE = NT_LOC * TPT  # total edge tiles per core

    # ---- I/O ----
    xT = nc.dram_tensor("xT", [C, N], BF16, kind="ExternalInput")
    xT_own = nc.dram_tensor("xT_own", [C, NPC], BF16, kind="ExternalInput")
    x_own = nc.dram_tensor("x_own", [NPC, C], F32, kind="ExternalInput")
    xe = nc.dram_tensor("xe", [TE * P, C], BF16, kind="ExternalInput")
    WqT = nc.dram_tensor("WqT", [C, C], BF16, kind="ExternalInput")
    WkT = nc.dram_tensor("WkT", [C, C], BF16, kind="ExternalInput")
    WvT = nc.dram_tensor("WvT", [C, C], BF16, kind="ExternalInput")
    Wop4 = nc.dram_tensor("Wop4", [DH, H * C], BF16, kind="ExternalInput")
    Wl = nc.dram_tensor("Wl", [C, C], BF16, kind="ExternalInput")
    fcT = nc.dram_tensor("fcT", [C, OUTC], BF16, kind="ExternalInput")
    bq_pack = nc.dram_tensor("bq_pack", [P, 2], F32, kind="ExternalInput")
    bk_pack = nc.dram_tensor("bk_pack", [P, 2], F32, kind="ExternalInput")
    bv_rep = nc.dram_tensor("bv_rep", [P, C], F32, kind="ExternalInput")
    opb_rep = nc.dram_tensor("opb_rep", [P, C], F32, kind="ExternalInput")
    g_rep = nc.dram_tensor("g_rep", [P, C], F32, kind="ExternalInput")
    b_rep = nc.dram_tensor("b_rep", [P, C], F32, kind="ExternalInput")
    fcb_rep = nc.dram_tensor("fcb_rep", [P, OUTC], F32, kind="ExternalInput")
    alpha11 = nc.dram_tensor("alpha11", [1, 1], F32, kind="ExternalInput")
    iota_in = nc.dram_tensor("iota_in", [P, P], BF16, kind="ExternalInput")
    ident_in = nc.dram_tensor("ident_in", [P, P], BF16, kind="ExternalInput")
    ones_row_in = nc.dram_tensor("ones_row_in", [1, P], F32, kind="ExternalInput")
    col_adj = nc.dram_tensor("col_adj", [P, TE], I32, kind="ExternalInput")
    dr_edge = nc.dram_tensor("dr_edge", [P, TE], I32, kind="ExternalInput")
    d_own = nc.dram_tensor("d_own", [P, NT_LOC], I32, kind="ExternalInput")

    out = nc.dram_tensor("out", [NPC, OUTC], F32, kind="ExternalOutput")

    with tile.TileContext(nc) as tc:
        with tc.tile_pool(name="const", bufs=1) as const:
            # phase-2-critical loads issue first on the sync queue
            Wq_t = const.tile([P, 2 * C], BF16)
            nc.sync.dma_start(out=Wq_t[:].rearrange("p (c n) -> p c n", c=2), in_=WqT[:].rearrange("(c p) n -> p c n", p=P))
            Wk_t = const.tile([P, 2 * C], BF16)
            nc.sync.dma_start(out=Wk_t[:].rearrange("p (c n) -> p c n", c=2), in_=WkT[:].rearrange("(c p) n -> p c n", p=P))
            Wv_t = const.tile([P, 2 * C], BF16)
            nc.sync.dma_start(out=Wv_t[:].rearrange("p (c n) -> p c n", c=2), in_=WvT[:].rearrange("(c p) n -> p c n", p=P))
            xo = [const.tile([P, NPC], BF16, tag=f"xo{c}", name=f"xo{c}") for c in range(2)]
            for c in range(2):
                nc.sync.dma_start(out=xo[c][:], in_=xT_own[c * P:(c + 1) * P, :])
            xts = [const.tile([P, N], BF16, name=f"xts{c}") for c in range(2)]
            for c in range(2):
                nc.sync.dma_start(out=xts[c][:], in_=xT[c * P:(c + 1) * P, :])
            bq_t = const.tile([P, 2], F32)
            nc.sync.dma_start(out=bq_t[:], in_=bq_pack[:])
            bk_t = const.tile([P, 2], F32)
            nc.sync.dma_start(out=bk_t[:], in_=bk_pack[:])
            bv_t = const.tile([P, C], F32)
            nc.sync.dma_start(out=bv_t[:], in_=bv_rep[:])

            # ---- persistent constants ----
            iota_t = const.tile([P, P], BF16)
            nc.sync.dma_start(out=iota_t[:], in_=iota_in[:])
            ident_t = const.tile([P, P], BF16)
            nc.sync.dma_start(out=ident_t[:], in_=ident_in[:])
            ones_row_t = const.tile([1, P], F32)
            nc.sync.dma_start(out=ones_row_t[:], in_=ones_row_in[:])
            col_t = const.tile([P, TE], I32)
            nc.sync.dma_start(out=col_t[:], in_=col_adj[:])
            colf_t = const.tile([P, TE], F32)
            nc.vector.tensor_copy(out=colf_t[:], in_=col_t[:])
            expb_col = const.tile([P, 1], F32)
            nc.vector.memset(expb_col[:], EXP_BIAS)
            eps_col = const.tile([P, 1], F32)
            nc.vector.memset(eps_col[:], 1e-5)
            w_col = const.tile([P, 1], F32)

            # guarded rsqrt of integer degrees: d=0 -> 0, else 1/sqrt(d)
            rs_row = const.tile([P, TE], F32)
            s_own = const.tile([P, NT_LOC], F32)

            with tc.tile_pool(name="ph1", bufs=1) as ph1, \
                 tc.tile_pool(name="ps1", bufs=1, space="PSUM") as ps1:
                al_t = ph1.tile([1, 1], F32)
                nc.sync.dma_start(out=al_t[:], in_=alpha11[:])
                wsig = ph1.tile([1, 1], F32)
                nc.scalar.activation(out=wsig[:], in_=al_t[:], func=AF.Sigmoid)
                wrep_ps = ps1.tile([P, 1], F32, tag="wrep")
                nc.tensor.matmul(out=wrep_ps[:], lhsT=ones_row_t[:], rhs=wsig[:],
                                 start=True, stop=True)
                nc.vector.tensor_copy(out=w_col[:], in_=wrep_ps[:])

                for (src_dram, dst, w_) in ((dr_edge, rs_row, TE), (d_own, s_own, NT_LOC)):
                    di = ph1.tile([P, w_], I32, tag=f"di{w_}")
                    nc.sync.dma_start(out=di[:], in_=src_dram[:])
                    df = ph1.tile([P, w_], F32, tag=f"df{w_}")
                    nc.vector.tensor_copy(out=df[:], in_=di[:])
                    m_t = ph1.tile([P, w_], F32, tag=f"m{w_}")
                    nc.vector.tensor_scalar(out=m_t[:], in0=df[:], scalar1=1.0,
                                            scalar2=None, op0=OP.min)
                    t1 = ph1.tile([P, w_], F32, tag=f"t1{w_}")
                    nc.vector.tensor_scalar(out=t1[:], in0=df[:], scalar1=1.0,
                                            scalar2=None, op0=OP.add)
                    nc.vector.tensor_tensor(out=t1[:], in0=t1[:], in1=m_t[:],
                                            op=OP.subtract)
                    nc.scalar.activation(out=t1[:], in_=t1[:], func=AF.Sqrt)
                    nc.vector.reciprocal(out=t1[:], in_=t1[:])
                    nc.vector.tensor_tensor(out=dst[:], in0=t1[:], in1=m_t[:],
                                            op=OP.mult)

            # ================= phase 2: QKV projections (bf16) =================
            big = const
            KTp = [big.tile([P, N], BF16, name=f"KT{p}") for p in range(2)]
            QTp = [big.tile([P, NPC], BF16, name=f"QT{p}") for p in range(2)]
            Vt = big.tile([P, NT_GLOB * H * (DH + 1)], BF16, name="Vt")
            V4 = Vt[:].rearrange("p (k h d) -> p k h d", h=H, d=DH + 1)
            OTu_sb = [big.tile([DH + 1, NPC], BF16, name=f"OTu{h}") for h in range(H)]
            hi_sb = [big.tile([P, C], BF16, name=f"hi{i}") for i in range(NT_LOC)]

            nc.vector.memset(V4[:, :, :, DH:DH + 1], 1.0)  # ones column for denominators

            with tc.tile_pool(name="ph2", bufs=1) as ph2, \
                 tc.tile_pool(name="ps2", bufs=1, space="PSUM") as ps2:
                for p in range(2):
                    qps = ps2.tile([P, NPC], F32, tag="qkps", bufs=2)
                    for c in range(2):
                        for nb in range(NPC // 512):
                            mi = nc.tensor.matmul(
                                out=qps[:, nb * 512:(nb + 1) * 512],
                                lhsT=Wq_t[:, c * C + p * P: c * C + (p + 1) * P],
                                rhs=xo[c][:, nb * 512:(nb + 1) * 512],
                                start=(c == 0), stop=(c == 1))
                            if nb > 0:
                                mi.ins.ldweights = False  # same weight chunk
                    nc.scalar.activation(
                        out=QTp[p][:], in_=qps[:],
                        func=AF.Identity, bias=bq_t[:, p:p + 1])

                # K and V in slabs of 1024 nodes from the two resident xT halves
                SLAB = 1024
                for s in range(N // SLAB):
                    for p in range(2):
                        kps = ps2.tile([P, SLAB], F32, tag="qkps", bufs=2)
                        for c in range(2):
                            for nb in range(SLAB // 512):
                                nc.tensor.matmul(
                                    out=kps[:, nb * 512:(nb + 1) * 512],
                                    lhsT=Wk_t[:, c * C + p * P: c * C + (p + 1) * P],
                                    rhs=xts[c][:, s * SLAB + nb * 512:s * SLAB + (nb + 1) * 512],
                                    start=(c == 0), stop=(c == 1))
                        nc.scalar.activation(
                            out=KTp[p][:, s * SLAB:(s + 1) * SLAB],
                            in_=kps[:], func=AF.Identity, bias=bk_t[:, p:p + 1])
                    for ntl in range(SLAB // P):
                        g = s * (SLAB // P) + ntl
                        vps = ps2.tile([P, C], F32, tag="vps", bufs=2)
                        for c in range(2):
                            nc.tensor.matmul(
                                out=vps[:],
                                lhsT=xts[c][:, g * P:(g + 1) * P],
                                rhs=Wv_t[:, c * C:(c + 1) * C],
                                start=(c == 0), stop=(c == 1))
                        nc.vector.tensor_tensor(
                            out=V4[:, g, :, 0:DH],
                            in0=vps[:].rearrange("p (h d) -> p h d", d=DH),
                            in1=bv_t[:].rearrange("p (h d) -> p h d", d=DH),
                            op=OP.add)

            # ========== phase 3: attention + interleaved GCN scatter ==========
            with tc.tile_pool(name="ph3", bufs=1) as ph3, \
                 tc.tile_pool(name="ps3", bufs=1, space="PSUM") as ps3:

                # GCN scatter jobs, interleaved across attention steps so the
                # sequential xe DMA streams overlap attention compute.  xe is
                # loaded XB tiles per DMA to keep the SP queue issue rate low.
                XB = 8
                scat_jobs = [(t, i) for t in range(NT_LOC) for i in range(TPT)]
                n_jobs = len(scat_jobs)
                n_steps = H * NT_GLOB
                emitted = 0
                hips_cur = {}
                xeb_cur = [None]

                built = 0
                oh_ring = {}

                def emit_scatter_builds(upto):
                    # one-hot builds (DVE) and xe loads run AHEAD of their
                    # consuming matmuls so the PE never waits on fresh data.
                    nonlocal built
                    while built < min(upto, n_jobs):
                        j = built
                        if j % XB == 0:
                            nb = min(XB, TE - j)
                            xeb_cur[0] = ph3.tile([P, XB, C], BF16, tag="xet", bufs=3,
                                                  name=f"xeb{j}")
                            nc.sync.dma_start(
                                out=xeb_cur[0][:, 0:nb, :],
                                in_=xe[j * P:(j + nb) * P, :].rearrange(
                                    "(i p) c -> p i c", p=P))
                        # weighted one-hot: (iota == col) * rsqrt(d_row)
                        oh = ph3.tile([P, P], BF16, tag="oh2", bufs=10, name=f"oh{j}")
                        nc.vector.tensor_scalar(
                            out=oh[:], in0=iota_t[:], scalar1=colf_t[:, j:j + 1],
                            scalar2=rs_row[:, j:j + 1], op0=OP.is_equal, op1=OP.mult)
                        oh_ring[j] = (oh, xeb_cur[0])
                        built += 1

                def emit_scatter_jobs(upto):
                    nonlocal emitted
                    emit_scatter_builds(upto + 6)
                    while emitted < min(upto, n_jobs):
                        t, i = scat_jobs[emitted]
                        j = t * TPT + i
                        if i == 0:
                            hips_cur[t] = ps3.tile([P, C], F32, tag="hips", bufs=2, name=f"hips{t}")
                        oh, xeb = oh_ring.pop(j)
                        nc.tensor.matmul(out=hips_cur[t][:], lhsT=oh[:],
                                         rhs=xeb[:, j % XB, :],
                                         start=(i == 0), stop=(i == TPT - 1))
                        if i == TPT - 1:
                            nc.vector.tensor_scalar(out=hi_sb[t][:], in0=hips_cur[t][:],
                                                    scalar1=s_own[:, t:t + 1],
                                                    scalar2=None, op0=OP.mult)
                        emitted += 1

                for h in range(H):
                    p, hh = h // 2, h % 2
                    po = hh * DH
                    OT_ps = ps3.tile([DH + 1, NPC], F32, tag="OT", bufs=1, name=f"OTps{h}")

                    def emit_attnv(kt, et):
                        for qh in range(2):
                            mi = nc.tensor.matmul(
                                out=OT_ps[:, qh * 512:(qh + 1) * 512],
                                lhsT=V4[:, kt, h, :],
                                rhs=et[:, qh * 512:(qh + 1) * 512],
                                start=(kt == 0), stop=(kt == NT_GLOB - 1))
                            if qh == 1:
                                mi.ins.ldweights = False  # same V tile as qh=0

                    # software-pipelined: attn@V runs two steps behind exp, so
                    # the PE consumes exp outputs produced ~2.5 us earlier and
                    # never waits on semaphore propagation from the ACT engine.
                    DEPTH = 2
                    et_hist = []
                    for kt in range(NT_GLOB):
                        sps = ps3.tile([P, NPC], F32, tag="sps", bufs=2)
                        for qh in range(2):
                            nc.tensor.matmul(
                                out=sps[:, qh * 512:(qh + 1) * 512],
                                lhsT=KTp[p][po:po + DH, kt * P:(kt + 1) * P],
                                rhs=QTp[p][po:po + DH, qh * 512:(qh + 1) * 512],
                                start=True, stop=True)
                        et = ph3.tile([P, NPC], BF16, tag="expT", bufs=DEPTH + 2)
                        nc.scalar.activation(out=et[:], in_=sps[:], func=AF.Exp,
                                             bias=expb_col[:, 0:1], scale=1.0 / np.sqrt(DH))
                        et_hist.append(et)
                        step = h * NT_GLOB + kt + 1
                        emit_scatter_jobs(n_jobs * step // n_steps)
                        if kt >= DEPTH:
                            emit_attnv(kt - DEPTH, et_hist[kt - DEPTH])
                    for kt in range(NT_GLOB - DEPTH, NT_GLOB):
                        emit_attnv(kt, et_hist[kt])

                    # drain PSUM (releases the accumulator); normalization by
                    # the denominator happens at the start of phase 4.
                    nc.vector.tensor_copy(out=OTu_sb[h][:], in_=OT_ps[:])

            # ================= phase 4: out_proj, LN, combine, fc =================
            with tc.tile_pool(name="ph4", bufs=1) as ph4:
                Wop_t = ph4.tile([DH, H * C], BF16)
                nc.sync.dma_start(out=Wop_t[:], in_=Wop4[:])
                Wl_t = ph4.tile([P, 2 * C], BF16)
                nc.sync.dma_start(out=Wl_t[:].rearrange("p (c n) -> p c n", c=2), in_=Wl[:].rearrange("(c p) n -> p c n", p=P))
                fc_t = ph4.tile([P, 2 * OUTC], BF16)
                nc.sync.dma_start(out=fc_t[:].rearrange("p (c n) -> p c n", c=2), in_=fcT[:].rearrange("(c p) n -> p c n", p=P))
                opb_t = ph4.tile([P, C], F32)
                nc.sync.dma_start(out=opb_t[:], in_=opb_rep[:])
                g_t = ph4.tile([P, C], F32)
                nc.sync.dma_start(out=g_t[:], in_=g_rep[:])
                b_t = ph4.tile([P, C], F32)
                nc.sync.dma_start(out=b_t[:], in_=b_rep[:])
                fcb_t = ph4.tile([P, OUTC], F32)
                nc.sync.dma_start(out=fcb_t[:], in_=fcb_rep[:])

                # ---- softmax denominators: transpose to node-major via tiny
                # ones-matmuls, single cheap reciprocal; 1/den is applied
                # per-query inside the qt loop below.
                with tc.tile_pool(name="ps4a", bufs=1, space="PSUM") as ps4a:
                    on64 = ph4.tile([P, 1], BF16)
                    nc.vector.memset(on64[:], 1.0)
                    den_cols = ph4.tile([P, H * NT_LOC], F32)
                    for qt in range(NT_LOC):
                        for h in range(H):
                            tpc = ps4a.tile([P, 1], F32, tag="tpc", bufs=2)
                            nc.tensor.matmul(
                                out=tpc[:],
                                lhsT=OTu_sb[h][DH:DH + 1, qt * P:(qt + 1) * P],
                                rhs=on64[64:65, 0:1],
                                start=True, stop=True)
                            nc.vector.tensor_copy(
                                out=den_cols[:, qt * H + h:qt * H + h + 1], in_=tpc[:])
                    rden_cols = ph4.tile([P, H * NT_LOC], F32)
                    nc.vector.reciprocal(out=rden_cols[:], in_=den_cols[:])

                ps4 = tc.alloc_tile_pool(name="ps4", bufs=1, space="PSUM")

                def transpose_2chunks(src_ap, tag, on_act=False):
                    dst = ph4.tile([P, C], BF16, tag=tag, bufs=2, name=f"t2c{tag}")
                    for c in range(2):
                        tp = ps4.tile([P, P], BF16, tag="tp", bufs=2)
                        nc.tensor.transpose(out=tp[:], in_=src_ap[:, c * P:(c + 1) * P],
                                            identity=ident_t[:])
                        if on_act:
                            nc.scalar.copy(out=dst[:, c * P:(c + 1) * P], in_=tp[:])
                        else:
                            nc.vector.tensor_copy(out=dst[:, c * P:(c + 1) * P], in_=tp[:])
                    return dst

                for qt in range(NT_LOC):
                    # ---- global path: per-head out_proj, scaled by 1/den per
                    # query while summing heads (ACT/DVE split) ----
                    xo_t = ph4.tile([P, C], F32, tag="xot", bufs=2)
                    nc.sync.dma_start(out=xo_t[:], in_=x_own[qt * P:(qt + 1) * P, :])
                    cps = [ps4.tile([P, C], F32, tag="cps", bufs=4, name=f"cps{h}_{qt}")
                           for h in range(H)]
                    for h in range(H):
                        nc.tensor.matmul(
                            out=cps[h][:],
                            lhsT=OTu_sb[h][0:DH, qt * P:(qt + 1) * P],
                            rhs=Wop_t[:, h * C:(h + 1) * C],
                            start=True, stop=True)
                    vs = [ph4.tile([P, C], F32, tag="vs", bufs=8, name=f"vs{h}_{qt}")
                          for h in range(H)]
                    for h in range(H):
                        r_ap = rden_cols[:, qt * H + h:qt * H + h + 1]
                        if h % 2 == 0:
                            nc.scalar.activation(out=vs[h][:], in_=cps[h][:],
                                                 func=AF.Copy, scale=r_ap)
                        else:
                            nc.vector.tensor_scalar(out=vs[h][:], in0=cps[h][:],
                                                    scalar1=r_ap, scalar2=None,
                                                    op0=OP.mult)
                    xob = ph4.tile([P, C], F32, tag="xob", bufs=2)
                    nc.vector.tensor_tensor(out=xob[:], in0=xo_t[:], in1=opb_t[:],
                                            op=OP.add)
                    nc.vector.tensor_tensor(out=vs[0][:], in0=vs[0][:], in1=vs[1][:],
                                            op=OP.add)
                    nc.gpsimd.tensor_tensor(out=vs[2][:], in0=vs[2][:], in1=vs[3][:],
                                            op=OP.add)
                    nc.vector.tensor_tensor(out=vs[0][:], in0=vs[0][:], in1=vs[2][:],
                                            op=OP.add)
                    v_t = ph4.tile([P, C], F32, tag="vt", bufs=2)
                    nc.vector.tensor_tensor(out=v_t[:], in0=vs[0][:], in1=xob[:],
                                            op=OP.add)
                    msum = ph4.tile([P, 1], F32, tag="msum", bufs=2)
                    nc.vector.reduce_sum(out=msum[:], in_=v_t[:], axis=AX.X)
                    mean_neg = ph4.tile([P, 1], F32, tag="mean", bufs=2)
                    nc.vector.tensor_scalar(out=mean_neg[:], in0=msum[:],
                                            scalar1=-1.0 / C, scalar2=None, op0=OP.mult)
                    # mean subtraction folded into Square's bias and the
                    # normalize step's bias; no standalone subtract pass.
                    sq = ph4.tile([P, C], F32, tag="sq", bufs=2)
                    ssum = ph4.tile([P, 1], F32, tag="ssum", bufs=2)
                    nc.scalar.activation(out=sq[:], in_=v_t[:], func=AF.Square,
                                         bias=mean_neg[:, 0:1], accum_out=ssum[:])
                    sstd = ph4.tile([P, 1], F32, tag="sstd", bufs=2)
                    nc.scalar.activation(out=sstd[:], in_=ssum[:], func=AF.Sqrt,
                                         bias=eps_col[:, 0:1], scale=1.0 / C)
                    rstd = ph4.tile([P, 1], F32, tag="rstd", bufs=2)
                    nc.vector.reciprocal(out=rstd[:], in_=sstd[:])
                    mnr = ph4.tile([P, 1], F32, tag="mnr", bufs=2)
                    nc.vector.tensor_tensor(out=mnr[:], in0=mean_neg[:], in1=rstd[:],
                                            op=OP.mult)
                    nc.scalar.activation(out=v_t[:], in_=v_t[:], func=AF.Identity,
                                         bias=mnr[:, 0:1], scale=rstd[:, 0:1])
                    nc.vector.tensor_tensor(out=v_t[:], in0=v_t[:], in1=g_t[:], op=OP.mult)
                    nc.vector.tensor_tensor(out=v_t[:], in0=v_t[:], in1=b_t[:], op=OP.add)

                    # ---- local path: local_embed = hi @ W_local ----
                    hiT = transpose_2chunks(hi_sb[qt][:], "hiT", on_act=True)
                    lps = ps4.tile([P, C], F32, tag="lps", bufs=2)
                    for c in range(2):
                        nc.tensor.matmul(out=lps[:], lhsT=hiT[:, c * P:(c + 1) * P],
                                         rhs=Wl_t[:, c * C:(c + 1) * C],
                                         start=(c == 0), stop=(c == 1))
                    # combined = global + w * (local - global)
                    comb = ph4.tile([P, C], F32, tag="comb", bufs=2)
                    nc.vector.tensor_tensor(out=comb[:], in0=lps[:], in1=v_t[:],
                                            op=OP.subtract)
                    nc.scalar.activation(out=comb[:], in_=comb[:], func=AF.Copy,
                                         scale=w_col[:, 0:1])
                    nc.vector.tensor_tensor(out=comb[:], in0=comb[:], in1=v_t[:], op=OP.add)
                    combb = ph4.tile([P, C], BF16, tag="combb", bufs=2)
                    nc.gpsimd.tensor_copy(out=combb[:], in_=comb[:])

                    # ---- fc ----
                    cT = transpose_2chunks(combb[:], "cT", on_act=True)
                    fps = ps4.tile([P, OUTC], F32, tag="lps", bufs=2, name=f"fps{qt}")
                    for c in range(2):
                        nc.tensor.matmul(out=fps[:], lhsT=cT[:, c * P:(c + 1) * P],
                                         rhs=fc_t[:, c * OUTC:(c + 1) * OUTC],
                                         start=(c == 0), stop=(c == 1))
                    o_t = ph4.tile([P, OUTC], F32, tag="ot", bufs=2)
                    nc.vector.tensor_tensor(out=o_t[:], in0=fps[:], in1=fcb_t[:], op=OP.add)
                    nc.sync.dma_start(out=out[qt * P:(qt + 1) * P, :], in_=o_t[:])
                ps4.release()
    nc.finalize()
    return nc


def _prep_edges(adj):
    """Bucket edges by destination node-tile; pad segments to a common length.

    Pure relayout/counting on the host: per-core local col indices, per-edge
    source-row indices (used to gather x into edge order), integer degrees.
    """
    row = np.asarray(adj[0], dtype=np.int64)
    col = np.asarray(adj[1], dtype=np.int64)
    d = np.bincount(col, minlength=N).astype(np.int32)
    tid = col // P
    order = np.argsort(tid, kind='stable')
    row_s, col_s = row[order], col[order]
    counts = np.bincount(tid, minlength=NT_GLOB)
    S = int(np.ceil(max(counts.max(), 1) / P) * P)
    TPT = S // P
    col_pad = np.full((NT_GLOB, S), -1, dtype=np.int32)
    row_pad = np.zeros((NT_GLOB, S), dtype=np.int32)
    start = 0
    for g in range(NT_GLOB):
        cnt = int(counts[g])
        col_pad[g, :cnt] = (col_s[start:start + cnt] - g * P).astype(np.int32)
        row_pad[g, :cnt] = row_s[start:start + cnt].astype(np.int32)
        start += cnt
    TE = NT_LOC * TPT
    per_core = []
    for k in range(NCORES):
        cols_k = col_pad[NT_LOC * k:NT_LOC * (k + 1)].reshape(TE, P)
        rows_k = row_pad[NT_LOC * k:NT_LOC * (k + 1)].reshape(TE, P)
        ca = np.ascontiguousarray(cols_k.T)                    # [P, TE]
        dre = np.ascontiguousarray(d[rows_k].T)                # [P, TE]
        down = np.ascontiguousarray(
            d[k * NPC:(k + 1) * NPC].reshape(NT_LOC, P).T)     # [P, NT_LOC]
        per_core.append((ca, rows_k.reshape(-1), dre, down))
    return per_core, TPT


def kernel(x, adj, weight_local, in_proj_w, in_proj_b, out_proj_w, out_proj_b,
           ln_g, ln_b, alpha, fc_w, fc_b):
    global LAST_RESULTS
    x = np.ascontiguousarray(np.asarray(x, dtype=np.float32))
    per_core_edges, TPT = _prep_edges(np.asarray(adj))

    bf = ml_dtypes.bfloat16
    x_bf = x.astype(bf)
    xT_bf = np.ascontiguousarray(x_bf.T)
    WopT = np.asarray(out_proj_w).T.astype(np.float32)  # [C_in, C_out]
    Wop4 = np.ascontiguousarray(
        WopT.reshape(H, DH, C).transpose(1, 0, 2).reshape(DH, H * C).astype(bf))
    common = dict(
        xT=xT_bf,
        WqT=np.ascontiguousarray(np.asarray(in_proj_w)[0:C].T.astype(bf)),
        WkT=np.ascontiguousarray(np.asarray(in_proj_w)[C:2 * C].T.astype(bf)),
        WvT=np.ascontiguousarray(np.asarray(in_proj_w)[2 * C:3 * C].T.astype(bf)),
        Wop4=Wop4,
        Wl=np.ascontiguousarray(np.asarray(weight_local).astype(bf)),
        fcT=np.ascontiguousarray(np.asarray(fc_w).T.astype(bf)),
        bq_pack=np.ascontiguousarray(np.asarray(in_proj_b)[0:C].astype(np.float32).reshape(2, P).T),
        bk_pack=np.ascontiguousarray(np.asarray(in_proj_b)[C:2 * C].astype(np.float32).reshape(2, P).T),
        bv_rep=np.tile(np.asarray(in_proj_b)[2 * C:3 * C].astype(np.float32), (P, 1)),
        opb_rep=np.tile(np.asarray(out_proj_b, dtype=np.float32), (P, 1)),
        g_rep=np.tile(np.asarray(ln_g, dtype=np.float32), (P, 1)),
        b_rep=np.tile(np.asarray(ln_b, dtype=np.float32), (P, 1)),
        fcb_rep=np.tile(np.asarray(fc_b, dtype=np.float32), (P, 1)),
        alpha11=np.asarray(alpha, dtype=np.float32).reshape(1, 1),
        iota_in=np.tile(np.arange(P, dtype=np.float32), (P, 1)).astype(bf),
        ident_in=np.eye(P, dtype=np.float32).astype(bf),
        ones_row_in=np.ones((1, P), dtype=np.float32),
    )
    in_maps = []
    for k in range(NCORES):
        ca, rows_flat, dre, down = per_core_edges[k]
        m = dict(common)
        m['xT_own'] = np.ascontiguousarray(xT_bf[:, k * NPC:(k + 1) * NPC])
        m['x_own'] = np.ascontiguousarray(x[k * NPC:(k + 1) * NPC, :])
        m['xe'] = np.ascontiguousarray(x_bf[rows_flat])
        m['col_adj'] = ca
        m['dr_edge'] = dre
        m['d_own'] = down
        in_maps.append(m)

    nc = _build(TPT)
    res = run_bass_kernel_spmd(nc, in_maps, core_ids=list(range(NCORES)))
    LAST_RESULTS = res
    return np.concatenate([res.results[k]['out'] for k in range(NCORES)], axis=0)


# revision 46
# speedup vs baseline: 1.0166x; 1.0166x over previous
"""Trainium2 Bass kernel for LocalGlobalEnvEncoder (GCN + MHA fusion).

Sharding: nodes are split across the 8 cores (1024 dest nodes / queries each).
 - GCN: edges bucketed by destination node-tile on host (layout only); source
   features are laid out in edge order on host (a pure gather / relayout), so
   the device streams them with plain sequential DMA instead of per-row
   indirect gathers. Per-edge 1/sqrt(d_row) weights are folded into the
   one-hot scatter matrix (built on DVE with a fused is_equal*mult chain) and
   the scatter-add runs on the PE in bf16.  Degrees are integer counts
   (host-side bincount relayout); all floating-point math (rsqrt, scaling,
   matmuls) happens on device.
 - MHA: query-sharded attention, K/V computed redundantly per core in bf16.
   Scores are kept transposed ([key, query]); exp runs on the ACT engine
   writing bf16; attn@V uses V as the stationary operand accumulating
   O^T [head_dim+1, queries] over key tiles, with softmax denominators coming
   from an appended ones-column in V.  Normalization (1/den) is applied
   column-wise via a gpsimd partition-broadcast.
All floating-point math happens on device; the host only re-lays-out inputs.
"""
import sys
sys.path.insert(0, '/opt/trn_rl_repo')
import numpy as np
import ml_dtypes
import concourse.bass as bass
import concourse.tile as tile
from concourse import bacc, mybir
from concourse.bass_utils import run_bass_kernel_spmd

F32 = mybir.dt.float32
BF16 = mybir.dt.bfloat16
I32 = mybir.dt.int32
AF = mybir.ActivationFunctionType
OP = mybir.AluOpType
AX = mybir.AxisListType

N, E, C, OUTC, H, DH = 8192, 262144, 256, 256, 4, 64
NCORES = 8
NPC = N // NCORES          # nodes per core = 1024
P = 128
NT_LOC = NPC // P          # node tiles per core = 8
NT_GLOB = N // P           # global node tiles = 64
EXP_BIAS = -12.0           # uniform shift inside softmax exp; cancels in the ratio

LAST_RESULTS = None        # stashed BassKernelResults for test harness introspection


def _build(TPT):
    """Build the single SPMD Bass program. TPT = edge tiles per node-tile segment."""
    nc = bacc.Bacc('TRN2', target_bir_lowering=False, debug=False, num_devices=NCORES)
    TE = NT_LOC * TPT  # total edge tiles per core

    # ---- I/O ----
    xT = nc.dram_tensor("xT", [C, N], BF16, kind="ExternalInput")
    xT_own = nc.dram_tensor("xT_own", [C, NPC], BF16, kind="ExternalInput")
    x_own = nc.dram_tensor("x_own", [NPC, C], F32, kind="ExternalInput")
    xe = nc.dram_tensor("xe", [TE * P, C], BF16, kind="ExternalInput")
    WqT = nc.dram_tensor("WqT", [C, C], BF16, kind="ExternalInput")
    WkT = nc.dram_tensor("WkT", [C, C], BF16, kind="ExternalInput")
    WvT = nc.dram_tensor("WvT", [C, C], BF16, kind="ExternalInput")
    Wop4 = nc.dram_tensor("Wop4", [DH, H * C], BF16, kind="ExternalInput")
    Wl = nc.dram_tensor("Wl", [C, C], BF16, kind="ExternalInput")
    fcT = nc.dram_tensor("fcT", [C, OUTC], BF16, kind="ExternalInput")
    bq_pack = nc.dram_tensor("bq_pack", [P, 2], F32, kind="ExternalInput")
    bk_pack = nc.dram_tensor("bk_pack", [P, 2], F32, kind="ExternalInput")
    bv_rep = nc.dram_tensor("bv_rep", [P, C], F32, kind="ExternalInput")
    opb_rep = nc.dram_tensor("opb_rep", [P, C], F32, kind="ExternalInput")
    g_rep = nc.dram_tensor("g_rep", [P, C], F32, kind="ExternalInput")
    b_rep = nc.dram_tensor("b_rep", [P, C], F32, kind="ExternalInput")
    fcb_rep = nc.dram_tensor("fcb_rep", [P, OUTC], F32, kind="ExternalInput")
    alpha11 = nc.dram_tensor("alpha11", [1, 1], F32, kind="ExternalInput")
    iota_in = nc.dram_tensor("iota_in", [P, P], BF16, kind="ExternalInput")
    ident_in = nc.dram_tensor("ident_in", [P, P], BF16, kind="ExternalInput")
    ones_row_in = nc.dram_tensor("ones_row_in", [1, P], F32, kind="ExternalInput")
    col_adj = nc.dram_tensor("col_adj", [P, TE], I32, kind="ExternalInput")
    dr_edge = nc.dram_tensor("dr_edge", [P, TE], I32, kind="ExternalInput")
    d_own = nc.dram_tensor("d_own", [P, NT_LOC], I32, kind="ExternalInput")

    out = nc.dram_tensor("out", [NPC, OUTC], F32, kind="ExternalOutput")

    with tile.TileContext(nc) as tc:
        with tc.tile_pool(name="const", bufs=1) as const:
            # phase-2-critical loads issue first on the sync queue
            Wq_t = const.tile([P, 2 * C], BF16)
            nc.sync.dma_start(out=Wq_t[:].rearrange("p (c n) -> p c n", c=2), in_=WqT[:].rearrange("(c p) n -> p c n", p=P))
            Wk_t = const.tile([P, 2 * C], BF16)
            nc.sync.dma_start(out=Wk_t[:].rearrange("p (c n) -> p c n", c=2), in_=WkT[:].rearrange("(c p) n -> p c n", p=P))
            Wv_t = const.tile([P, 2 * C], BF16)
            nc.sync.dma_start(out=Wv_t[:].rearrange("p (c n) -> p c n", c=2), in_=WvT[:].rearrange("(c p) n -> p c n", p=P))
            xo = [const.tile([P, NPC], BF16, tag=f"xo{c}", name=f"xo{c}") for c in range(2)]
            for c in range(2):
                nc.sync.dma_start(out=xo[c][:], in_=xT_own[c * P:(c + 1) * P, :])
            xts = [const.tile([P, N], BF16, name=f"xts{c}") for c in range(2)]
            for c in range(2):
                nc.sync.dma_start(out=xts[c][:], in_=xT[c * P:(c + 1) * P, :])
            bq_t = const.tile([P, 2], F32)
            nc.sync.dma_start(out=bq_t[:], in_=bq_pack[:])
            bk_t = const.tile([P, 2], F32)
            nc.sync.dma_start(out=bk_t[:], in_=bk_pack[:])
            bv_t = const.tile([P, C], F32)
            nc.sync.dma_start(out=bv_t[:], in_=bv_rep[:])

            # ---- persistent constants ----
            iota_t = const.tile([P, P], BF16)
            nc.sync.dma_start(out=iota_t[:], in_=iota_in[:])
            ident_t = const.tile([P, P], BF16)
            nc.sync.dma_start(out=ident_t[:], in_=ident_in[:])
            ones_row_t = const.tile([1, P], F32)
            nc.sync.dma_start(out=ones_row_t[:], in_=ones_row_in[:])
            col_t = const.tile([P, TE], I32)
            nc.sync.dma_start(out=col_t[:], in_=col_adj[:])
            colf_t = const.tile([P, TE], F32)
            nc.vector.tensor_copy(out=colf_t[:], in_=col_t[:])
            expb_col = const.tile([P, 1], F32)
            nc.vector.memset(expb_col[:], EXP_BIAS)
            eps_col = const.tile([P, 1], F32)
            nc.vector.memset(eps_col[:], 1e-5)
            w_col = const.tile([P, 1], F32)

            # guarded rsqrt of integer degrees: d=0 -> 0, else 1/sqrt(d)
            rs_row = const.tile([P, TE], F32)
            s_own = const.tile([P, NT_LOC], F32)

            with tc.tile_pool(name="ph1", bufs=1) as ph1, \
                 tc.tile_pool(name="ps1", bufs=1, space="PSUM") as ps1:
                al_t = ph1.tile([1, 1], F32)
                nc.sync.dma_start(out=al_t[:], in_=alpha11[:])
                wsig = ph1.tile([1, 1], F32)
                nc.scalar.activation(out=wsig[:], in_=al_t[:], func=AF.Sigmoid)
                wrep_ps = ps1.tile([P, 1], F32, tag="wrep")
                nc.tensor.matmul(out=wrep_ps[:], lhsT=ones_row_t[:], rhs=wsig[:],
                                 start=True, stop=True)
                nc.vector.tensor_copy(out=w_col[:], in_=wrep_ps[:])

                for (src_dram, dst, w_) in ((dr_edge, rs_row, TE), (d_own, s_own, NT_LOC)):
                    di = ph1.tile([P, w_], I32, tag=f"di{w_}")
                    nc.sync.dma_start(out=di[:], in_=src_dram[:])
                    df = ph1.tile([P, w_], F32, tag=f"df{w_}")
                    nc.vector.tensor_copy(out=df[:], in_=di[:])
                    m_t = ph1.tile([P, w_], F32, tag=f"m{w_}")
                    nc.vector.tensor_scalar(out=m_t[:], in0=df[:], scalar1=1.0,
                                            scalar2=None, op0=OP.min)
                    t1 = ph1.tile([P, w_], F32, tag=f"t1{w_}")
                    nc.vector.tensor_scalar(out=t1[:], in0=df[:], scalar1=1.0,
                                            scalar2=None, op0=OP.add)
                    nc.vector.tensor_tensor(out=t1[:], in0=t1[:], in1=m_t[:],
                                            op=OP.subtract)
                    nc.scalar.activation(out=t1[:], in_=t1[:], func=AF.Sqrt)
                    nc.vector.reciprocal(out=t1[:], in_=t1[:])
                    nc.vector.tensor_tensor(out=dst[:], in0=t1[:], in1=m_t[:],
                                            op=OP.mult)

            # ================= phase 2: QKV projections (bf16) =================
            big = const
            KTp = [big.tile([P, N], BF16, name=f"KT{p}") for p in range(2)]
            QTp = [big.tile([P, NPC], BF16, name=f"QT{p}") for p in range(2)]
            Vt = big.tile([P, NT_GLOB * H * (DH + 1)], BF16, name="Vt")
            V4 = Vt[:].rearrange("p (k h d) -> p k h d", h=H, d=DH + 1)
            OTu_sb = [big.tile([DH + 1, NPC], BF16, name=f"OTu{h}") for h in range(H)]
            hi_sb = [big.tile([P, C], BF16, name=f"hi{i}") for i in range(NT_LOC)]

            nc.vector.memset(V4[:, :, :, DH:DH + 1], 1.0)  # ones column for denominators

            with tc.tile_pool(name="ph2", bufs=1) as ph2, \
                 tc.tile_pool(name="ps2", bufs=1, space="PSUM") as ps2:
                for p in range(2):
                    qps = ps2.tile([P, NPC], F32, tag="qkps", bufs=2)
                    for c in range(2):
                        for nb in range(NPC // 512):
                            mi = nc.tensor.matmul(
                                out=qps[:, nb * 512:(nb + 1) * 512],
                                lhsT=Wq_t[:, c * C + p * P: c * C + (p + 1) * P],
                                rhs=xo[c][:, nb * 512:(nb + 1) * 512],
                                start=(c == 0), stop=(c == 1))
                            if nb > 0:
                                mi.ins.ldweights = False  # same weight chunk
                    nc.scalar.activation(
                        out=QTp[p][:], in_=qps[:],
                        func=AF.Identity, bias=bq_t[:, p:p + 1])

                # K and V in slabs of 1024 nodes from the two resident xT halves
                SLAB = 1024
                for s in range(N // SLAB):
                    for p in range(2):
                        kps = ps2.tile([P, SLAB], F32, tag="qkps", bufs=2)
                        for c in range(2):
                            for nb in range(SLAB // 512):
                                nc.tensor.matmul(
                                    out=kps[:, nb * 512:(nb + 1) * 512],
                                    lhsT=Wk_t[:, c * C + p * P: c * C + (p + 1) * P],
                                    rhs=xts[c][:, s * SLAB + nb * 512:s * SLAB + (nb + 1) * 512],
                                    start=(c == 0), stop=(c == 1))
                        nc.scalar.activation(
                            out=KTp[p][:, s * SLAB:(s + 1) * SLAB],
                            in_=kps[:], func=AF.Identity, bias=bk_t[:, p:p + 1])
                    for ntl in range(SLAB // P):
                        g = s * (SLAB // P) + ntl
                        vps = ps2.tile([P, C], F32, tag="vps", bufs=2)
                        for c in range(2):
                            nc.tensor.matmul(
                                out=vps[:],
                                lhsT=xts[c][:, g * P:(g + 1) * P],
                                rhs=Wv_t[:, c * C:(c + 1) * C],
                                start=(c == 0), stop=(c == 1))
                        nc.vector.tensor_tensor(
                            out=V4[:, g, :, 0:DH],
                            in0=vps[:].rearrange("p (h d) -> p h d", d=DH),
                            in1=bv_t[:].rearrange("p (h d) -> p h d", d=DH),
                            op=OP.add)

            # ========== phase 3: attention + interleaved GCN scatter ==========
            with tc.tile_pool(name="ph3", bufs=1) as ph3, \
                 tc.tile_pool(name="ps3", bufs=1, space="PSUM") as ps3:

                # GCN scatter jobs, interleaved across attention steps so the
                # sequential xe DMA streams overlap attention compute.  xe is
                # loaded XB tiles per DMA to keep the SP queue issue rate low.
                XB = 8
                scat_jobs = [(t, i) for t in range(NT_LOC) for i in range(TPT)]
                n_jobs = len(scat_jobs)
                n_steps = H * NT_GLOB
                emitted = 0
                hips_cur = {}
                xeb_cur = [None]

                built = 0
                oh_ring = {}

                def emit_scatter_builds(upto):
                    # one-hot builds (DVE) and xe loads run AHEAD of their
                    # consuming matmuls so the PE never waits on fresh data.
                    nonlocal built
                    while built < min(upto, n_jobs):
                        j = built
                        if j % XB == 0:
                            nb = min(XB, TE - j)
                            xeb_cur[0] = ph3.tile([P, XB, C], BF16, tag="xet", bufs=3,
                                                  name=f"xeb{j}")
                            nc.sync.dma_start(
                                out=xeb_cur[0][:, 0:nb, :],
                                in_=xe[j * P:(j + nb) * P, :].rearrange(
                                    "(i p) c -> p i c", p=P))
                        # weighted one-hot: (iota == col) * rsqrt(d_row)
                        oh = ph3.tile([P, P], BF16, tag="oh2", bufs=10, name=f"oh{j}")
                        nc.vector.tensor_scalar(
                            out=oh[:], in0=iota_t[:], scalar1=colf_t[:, j:j + 1],
                            scalar2=rs_row[:, j:j + 1], op0=OP.is_equal, op1=OP.mult)
                        oh_ring[j] = (oh, xeb_cur[0])
                        built += 1

                def emit_scatter_jobs(upto):
                    nonlocal emitted
                    emit_scatter_builds(upto + 6)
                    while emitted < min(upto, n_jobs):
                        t, i = scat_jobs[emitted]
                        j = t * TPT + i
                        if i == 0:
                            hips_cur[t] = ps3.tile([P, C], F32, tag="hips", bufs=2, name=f"hips{t}")
                        oh, xeb = oh_ring.pop(j)
                        nc.tensor.matmul(out=hips_cur[t][:], lhsT=oh[:],
                                         rhs=xeb[:, j % XB, :],
                                         start=(i == 0), stop=(i == TPT - 1))
                        if i == TPT - 1:
                            nc.vector.tensor_scalar(out=hi_sb[t][:], in0=hips_cur[t][:],
                                                    scalar1=s_own[:, t:t + 1],
                                                    scalar2=None, op0=OP.mult)
                        emitted += 1

                for h in range(H):
                    p, hh = h // 2, h % 2
                    po = hh * DH
                    OT_ps = ps3.tile([DH + 1, NPC], F32, tag="OT", bufs=1, name=f"OTps{h}")

                    def emit_attnv(kt, et):
                        for qh in range(2):
                            mi = nc.tensor.matmul(
                                out=OT_ps[:, qh * 512:(qh + 1) * 512],
                                lhsT=V4[:, kt, h, :],
                                rhs=et[:, qh * 512:(qh + 1) * 512],
                                start=(kt == 0), stop=(kt == NT_GLOB - 1))
                            if qh == 1:
                                mi.ins.ldweights = False  # same V tile as qh=0

                    # software-pipelined: attn@V runs two steps behind exp, so
                    # the PE consumes exp outputs produced ~2.5 us earlier and
                    # never waits on semaphore propagation from the ACT engine.
                    DEPTH = 2
                    et_hist = []
                    for kt in range(NT_GLOB):
                        sps = ps3.tile([P, NPC], F32, tag="sps", bufs=2)
                        for qh in range(2):
                            nc.tensor.matmul(
                                out=sps[:, qh * 512:(qh + 1) * 512],
                                lhsT=KTp[p][po:po + DH, kt * P:(kt + 1) * P],
                                rhs=QTp[p][po:po + DH, qh * 512:(qh + 1) * 512],
                                start=True, stop=True)
                        et = ph3.tile([P, NPC], BF16, tag="expT", bufs=DEPTH + 2)
                        nc.scalar.activation(out=et[:], in_=sps[:], func=AF.Exp,
                                             bias=expb_col[:, 0:1], scale=1.0 / np.sqrt(DH))
                        et_hist.append(et)
                        step = h * NT_GLOB + kt + 1
                        emit_scatter_jobs(n_jobs * step // n_steps)
                        if kt >= DEPTH:
                            emit_attnv(kt - DEPTH, et_hist[kt - DEPTH])
                    for kt in range(NT_GLOB - DEPTH, NT_GLOB):
                        emit_attnv(kt, et_hist[kt])

                    # drain PSUM (releases the accumulator); normalization by
                    # the denominator happens at the start of phase 4.
                    nc.vector.tensor_copy(out=OTu_sb[h][:], in_=OT_ps[:])

            # ================= phase 4: out_proj, LN, combine, fc =================
            with tc.tile_pool(name="ph4", bufs=1) as ph4:
                Wop_t = ph4.tile([DH, H * C], BF16)
                nc.sync.dma_start(out=Wop_t[:], in_=Wop4[:])
                Wl_t = ph4.tile([P, 2 * C], BF16)
                nc.sync.dma_start(out=Wl_t[:].rearrange("p (c n) -> p c n", c=2), in_=Wl[:].rearrange("(c p) n -> p c n", p=P))
                fc_t = ph4.tile([P, 2 * OUTC], BF16)
                nc.sync.dma_start(out=fc_t[:].rearrange("p (c n) -> p c n", c=2), in_=fcT[:].rearrange("(c p) n -> p c n", p=P))
                opb_t = ph4.tile([P, C], F32)
                nc.sync.dma_start(out=opb_t[:], in_=opb_rep[:])
                g_t = ph4.tile([P, C], F32)
                nc.sync.dma_start(out=g_t[:], in_=g_rep[:])
                b_t = ph4.tile([P, C], F32)
                nc.sync.dma_start(out=b_t[:], in_=b_rep[:])
                fcb_t = ph4.tile([P, OUTC], F32)
                nc.sync.dma_start(out=fcb_t[:], in_=fcb_rep[:])

                # ---- softmax denominators: transpose to node-major via tiny
                # ones-matmuls, single cheap reciprocal; 1/den is applied
                # per-query inside the qt loop below.
                with tc.tile_pool(name="ps4a", bufs=1, space="PSUM") as ps4a:
                    on64 = ph4.tile([P, 1], BF16)
                    nc.vector.memset(on64[:], 1.0)
                    den_cols = ph4.tile([P, H * NT_LOC], F32)
                    for qt in range(NT_LOC):
                        for h in range(H):
                            tpc = ps4a.tile([P, 1], F32, tag="tpc", bufs=2)
                            nc.tensor.matmul(
                                out=tpc[:],
                                lhsT=OTu_sb[h][DH:DH + 1, qt * P:(qt + 1) * P],
                                rhs=on64[64:65, 0:1],
                                start=True, stop=True)
                            nc.vector.tensor_copy(
                                out=den_cols[:, qt * H + h:qt * H + h + 1], in_=tpc[:])
                    rden_cols = ph4.tile([P, H * NT_LOC], F32)
                    nc.vector.reciprocal(out=rden_cols[:], in_=den_cols[:])

                ps4 = tc.alloc_tile_pool(name="ps4", bufs=1, space="PSUM")

                def transpose_2chunks(src_ap, tag, on_act=False):
                    dst = ph4.tile([P, C], BF16, tag=tag, bufs=2, name=f"t2c{tag}")
                    for c in range(2):
                        tp = ps4.tile([P, P], BF16, tag="tp", bufs=2)
                        nc.tensor.transpose(out=tp[:], in_=src_ap[:, c * P:(c + 1) * P],
                                            identity=ident_t[:])
                        if on_act:
                            nc.scalar.copy(out=dst[:, c * P:(c + 1) * P], in_=tp[:])
                        else:
                            nc.vector.tensor_copy(out=dst[:, c * P:(c + 1) * P], in_=tp[:])
                    return dst

                for qt in range(NT_LOC):
                    # ---- global path: per-head out_proj, scaled by 1/den per
                    # query while summing heads (ACT/DVE split) ----
                    xo_t = ph4.tile([P, C], F32, tag="xot", bufs=2)
                    nc.sync.dma_start(out=xo_t[:], in_=x_own[qt * P:(qt + 1) * P, :])
                    cps = [ps4.tile([P, C], F32, tag="cps", bufs=4, name=f"cps{h}_{qt}")
                           for h in range(H)]
                    for h in range(H):
                        nc.tensor.matmul(
                            out=cps[h][:],
                            lhsT=OTu_sb[h][0:DH, qt * P:(qt + 1) * P],
                            rhs=Wop_t[:, h * C:(h + 1) * C],
                            start=True, stop=True)
                    vs = [ph4.tile([P, C], F32, tag="vs", bufs=8, name=f"vs{h}_{qt}")
                          for h in range(H)]
                    for h in range(H):
                        r_ap = rden_cols[:, qt * H + h:qt * H + h + 1]
                        if h % 2 == 0:
                            nc.scalar.activation(out=vs[h][:], in_=cps[h][:],
                                                 func=AF.Copy, scale=r_ap)
                        else:
                            nc.vector.tensor_scalar(out=vs[h][:], in0=cps[h][:],
                                                    scalar1=r_ap, scalar2=None,
                                                    op0=OP.mult)
                    xob = ph4.tile([P, C], F32, tag="xob", bufs=2)
                    nc.vector.tensor_tensor(out=xob[:], in0=xo_t[:], in1=opb_t[:],
                                            op=OP.add)
                    nc.vector.tensor_tensor(out=vs[0][:], in0=vs[0][:], in1=vs[1][:],
                                            op=OP.add)
                    nc.gpsimd.tensor_tensor(out=vs[2][:], in0=vs[2][:], in1=vs[3][:],
                                            op=OP.add)
                    nc.vector.tensor_tensor(out=vs[0][:], in0=vs[0][:], in1=vs[2][:],
                                            op=OP.add)
                    v_t = ph4.tile([P, C], F32, tag="vt", bufs=2)
                    nc.vector.tensor_tensor(out=v_t[:], in0=vs[0][:], in1=xob[:],
                                            op=OP.add)
                    msum = ph4.tile([P, 1], F32, tag="msum", bufs=2)
                    nc.vector.reduce_sum(out=msum[:], in_=v_t[:], axis=AX.X)
                    mean_neg = ph4.tile([P, 1], F32, tag="mean", bufs=2)
                    nc.vector.tensor_scalar(out=mean_neg[:], in0=msum[:],
                                            scalar1=-1.0 / C, scalar2=None, op0=OP.mult)
                    # mean subtraction folded into Square's bias and the
                    # normalize step's bias; no standalone subtract pass.
                    sq = ph4.tile([P, C], F32, tag="sq", bufs=2)
                    ssum = ph4.tile([P, 1], F32, tag="ssum", bufs=2)
                    nc.scalar.activation(out=sq[:], in_=v_t[:], func=AF.Square,
                                         bias=mean_neg[:, 0:1], accum_out=ssum[:])
                    sstd = ph4.tile([P, 1], F32, tag="sstd", bufs=2)
                    nc.scalar.activation(out=sstd[:], in_=ssum[:], func=AF.Sqrt,
                                         bias=eps_col[:, 0:1], scale=1.0 / C)
                    rstd = ph4.tile([P, 1], F32, tag="rstd", bufs=2)
                    nc.vector.reciprocal(out=rstd[:], in_=sstd[:])
                    mnr = ph4.tile([P, 1], F32, tag="mnr", bufs=2)
                    nc.vector.tensor_tensor(out=mnr[:], in0=mean_neg[:], in1=rstd[:],
                                            op=OP.mult)
                    nc.scalar.activation(out=v_t[:], in_=v_t[:], func=AF.Identity,
                                         bias=mnr[:, 0:1], scale=rstd[:, 0:1])
                    nc.vector.tensor_tensor(out=v_t[:], in0=v_t[:], in1=g_t[:], op=OP.mult)
                    nc.vector.tensor_tensor(out=v_t[:], in0=v_t[:], in1=b_t[:], op=OP.add)

                    # ---- local path: local_embed = hi @ W_local ----
                    hiT = transpose_2chunks(hi_sb[qt][:], "hiT", on_act=True)
                    lps = ps4.tile([P, C], F32, tag="lps", bufs=2)
                    for c in range(2):
                        nc.tensor.matmul(out=lps[:], lhsT=hiT[:, c * P:(c + 1) * P],
                                         rhs=Wl_t[:, c * C:(c + 1) * C],
                                         start=(c == 0), stop=(c == 1))
                    # combined = global + w * (local - global)
                    comb = ph4.tile([P, C], F32, tag="comb", bufs=2)
                    nc.vector.tensor_tensor(out=comb[:], in0=lps[:], in1=v_t[:],
                                            op=OP.subtract)
                    nc.scalar.activation(out=comb[:], in_=comb[:], func=AF.Copy,
                                         scale=w_col[:, 0:1])
                    nc.vector.tensor_tensor(out=comb[:], in0=comb[:], in1=v_t[:], op=OP.add)
                    combb = ph4.tile([P, C], BF16, tag="combb", bufs=2)
                    nc.scalar.copy(out=combb[:], in_=comb[:])

                    # ---- fc ----
                    cT = transpose_2chunks(combb[:], "cT", on_act=True)
                    fps = ps4.tile([P, OUTC], F32, tag="lps", bufs=2, name=f"fps{qt}")
                    for c in range(2):
                        nc.tensor.matmul(out=fps[:], lhsT=cT[:, c * P:(c + 1) * P],
                                         rhs=fc_t[:, c * OUTC:(c + 1) * OUTC],
                                         start=(c == 0), stop=(c == 1))
                    o_t = ph4.tile([P, OUTC], F32, tag="ot", bufs=2)
                    nc.vector.tensor_tensor(out=o_t[:], in0=fps[:], in1=fcb_t[:], op=OP.add)
                    nc.sync.dma_start(out=out[qt * P:(qt + 1) * P, :], in_=o_t[:])
                ps4.release()
    nc.finalize()
    return nc


def _prep_edges(adj):
    """Bucket edges by destination node-tile; pad segments to a common length.

    Pure relayout/counting on the host: per-core local col indices, per-edge
    source-row indices (used to gather x into edge order), integer degrees.
    """
    row = np.asarray(adj[0], dtype=np.int64)
    col = np.asarray(adj[1], dtype=np.int64)
    d = np.bincount(col, minlength=N).astype(np.int32)
    tid = col // P
    order = np.argsort(tid, kind='stable')
    row_s, col_s = row[order], col[order]
    counts = np.bincount(tid, minlength=NT_GLOB)
    S = int(np.ceil(max(counts.max(), 1) / P) * P)
    TPT = S // P
    col_pad = np.full((NT_GLOB, S), -1, dtype=np.int32)
    row_pad = np.zeros((NT_GLOB, S), dtype=np.int32)
    start = 0
    for g in range(NT_GLOB):
        cnt = int(counts[g])
        col_pad[g, :cnt] = (col_s[start:start + cnt] - g * P).astype(np.int32)
        row_pad[g, :cnt] = row_s[start:start + cnt].astype(np.int32)
        start += cnt
    TE = NT_LOC * TPT
    per_core = []
    for k in range(NCORES):
        cols_k = col_pad[NT_LOC * k:NT_LOC * (k + 1)].reshape(TE, P)
        rows_k = row_pad[NT_LOC * k:NT_LOC * (k + 1)].reshape(TE, P)
        ca = np.ascontiguousarray(cols_k.T)                    # [P, TE]
        dre = np.ascontiguousarray(d[rows_k].T)                # [P, TE]
        down = np.ascontiguousarray(
            d[k * NPC:(k + 1) * NPC].reshape(NT_LOC, P).T)     # [P, NT_LOC]
        per_core.append((ca, rows_k.reshape(-1), dre, down))
    return per_core, TPT


def kernel(x, adj, weight_local, in_proj_w, in_proj_b, out_proj_w, out_proj_b,
           ln_g, ln_b, alpha, fc_w, fc_b):
    global LAST_RESULTS
    x = np.ascontiguousarray(np.asarray(x, dtype=np.float32))
    per_core_edges, TPT = _prep_edges(np.asarray(adj))

    bf = ml_dtypes.bfloat16
    x_bf = x.astype(bf)
    xT_bf = np.ascontiguousarray(x_bf.T)
    WopT = np.asarray(out_proj_w).T.astype(np.float32)  # [C_in, C_out]
    Wop4 = np.ascontiguousarray(
        WopT.reshape(H, DH, C).transpose(1, 0, 2).reshape(DH, H * C).astype(bf))
    common = dict(
        xT=xT_bf,
        WqT=np.ascontiguousarray(np.asarray(in_proj_w)[0:C].T.astype(bf)),
        WkT=np.ascontiguousarray(np.asarray(in_proj_w)[C:2 * C].T.astype(bf)),
        WvT=np.ascontiguousarray(np.asarray(in_proj_w)[2 * C:3 * C].T.astype(bf)),
        Wop4=Wop4,
        Wl=np.ascontiguousarray(np.asarray(weight_local).astype(bf)),
        fcT=np.ascontiguousarray(np.asarray(fc_w).T.astype(bf)),
        bq_pack=np.ascontiguousarray(np.asarray(in_proj_b)[0:C].astype(np.float32).reshape(2, P).T),
        bk_pack=np.ascontiguousarray(np.asarray(in_proj_b)[C:2 * C].astype(np.float32).reshape(2, P).T),
        bv_rep=np.tile(np.asarray(in_proj_b)[2 * C:3 * C].astype(np.float32), (P, 1)),
        opb_rep=np.tile(np.asarray(out_proj_b, dtype=np.float32), (P, 1)),
        g_rep=np.tile(np.asarray(ln_g, dtype=np.float32), (P, 1)),
        b_rep=np.tile(np.asarray(ln_b, dtype=np.float32), (P, 1)),
        fcb_rep=np.tile(np.asarray(fc_b, dtype=np.float32), (P, 1)),
        alpha11=np.asarray(alpha, dtype=np.float32).reshape(1, 1),
        iota_in=np.tile(np.arange(P, dtype=np.float32), (P, 1)).astype(bf),
        ident_in=np.eye(P, dtype=np.float32).astype(bf),
        ones_row_in=np.ones((1, P), dtype=np.float32),
    )
    in_maps = []
    for k in range(NCORES):
        ca, rows_flat, dre, down = per_core_edges[k]
        m = dict(common)
        m['xT_own'] = np.ascontiguousarray(xT_bf[:, k * NPC:(k + 1) * NPC])
        m['x_own'] = np.ascontiguousarray(x[k * NPC:(k + 1) * NPC, :])
        m['xe'] = np.ascontiguousarray(x_bf[rows_flat])
        m['col_adj'] = ca
        m['dr_edge'] = dre
        m['d_own'] = down
        in_maps.append(m)

    nc = _build(TPT)
    res = run_bass_kernel_spmd(nc, in_maps, core_ids=list(range(NCORES)))
    LAST_RESULTS = res
    return np.concatenate([res.results[k]['out'] for k in range(NCORES)], axis=0)


# revision 47
# speedup vs baseline: 1.0182x; 1.0015x over previous
"""Trainium2 Bass kernel for LocalGlobalEnvEncoder (GCN + MHA fusion).

Sharding: nodes are split across the 8 cores (1024 dest nodes / queries each).
 - GCN: edges bucketed by destination node-tile on host (layout only); source
   features are laid out in edge order on host (a pure gather / relayout), so
   the device streams them with plain sequential DMA instead of per-row
   indirect gathers. Per-edge 1/sqrt(d_row) weights are folded into the
   one-hot scatter matrix (built on DVE with a fused is_equal*mult chain) and
   the scatter-add runs on the PE in bf16.  Degrees are integer counts
   (host-side bincount relayout); all floating-point math (rsqrt, scaling,
   matmuls) happens on device.
 - MHA: query-sharded attention, K/V computed redundantly per core in bf16.
   Scores are kept transposed ([key, query]); exp runs on the ACT engine
   writing bf16; attn@V uses V as the stationary operand accumulating
   O^T [head_dim+1, queries] over key tiles, with softmax denominators coming
   from an appended ones-column in V.  Normalization (1/den) is applied
   per-query in node-major layout during the output projection.
All floating-point math happens on device; the host only re-lays-out inputs.
"""
import sys
sys.path.insert(0, '/opt/trn_rl_repo')
import numpy as np
import ml_dtypes
import concourse.bass as bass
import concourse.tile as tile
from concourse import bacc, mybir
from concourse.bass_utils import run_bass_kernel_spmd

F32 = mybir.dt.float32
BF16 = mybir.dt.bfloat16
I32 = mybir.dt.int32
AF = mybir.ActivationFunctionType
OP = mybir.AluOpType
AX = mybir.AxisListType

N, E, C, OUTC, H, DH = 8192, 262144, 256, 256, 4, 64
NCORES = 8
NPC = N // NCORES          # nodes per core = 1024
P = 128
NT_LOC = NPC // P          # node tiles per core = 8
NT_GLOB = N // P           # global node tiles = 64
EXP_BIAS = -12.0           # uniform shift inside softmax exp; cancels in the ratio

LAST_RESULTS = None        # stashed BassKernelResults for test harness introspection


def _build(TPT):
    """Build the single SPMD Bass program. TPT = edge tiles per node-tile segment."""
    nc = bacc.Bacc('TRN2', target_bir_lowering=False, debug=False, num_devices=NCORES)
    TE = NT_LOC * TPT  # total edge tiles per core

    # ---- I/O ----
    xT = nc.dram_tensor("xT", [C, N], BF16, kind="ExternalInput")
    xT_own = nc.dram_tensor("xT_own", [C, NPC], BF16, kind="ExternalInput")
    x_own = nc.dram_tensor("x_own", [NPC, C], F32, kind="ExternalInput")
    xe = nc.dram_tensor("xe", [TE * P, C], BF16, kind="ExternalInput")
    WqT = nc.dram_tensor("WqT", [C, C], BF16, kind="ExternalInput")
    WkT = nc.dram_tensor("WkT", [C, C], BF16, kind="ExternalInput")
    WvT = nc.dram_tensor("WvT", [C, C], BF16, kind="ExternalInput")
    Wop4 = nc.dram_tensor("Wop4", [DH, H * C], BF16, kind="ExternalInput")
    Wl = nc.dram_tensor("Wl", [C, C], BF16, kind="ExternalInput")
    fcT = nc.dram_tensor("fcT", [C, OUTC], BF16, kind="ExternalInput")
    bq_pack = nc.dram_tensor("bq_pack", [P, 2], F32, kind="ExternalInput")
    bk_pack = nc.dram_tensor("bk_pack", [P, 2], F32, kind="ExternalInput")
    bv_rep = nc.dram_tensor("bv_rep", [P, C], F32, kind="ExternalInput")
    opb_rep = nc.dram_tensor("opb_rep", [P, C], F32, kind="ExternalInput")
    g_rep = nc.dram_tensor("g_rep", [P, C], F32, kind="ExternalInput")
    b_rep = nc.dram_tensor("b_rep", [P, C], F32, kind="ExternalInput")
    fcb_rep = nc.dram_tensor("fcb_rep", [P, OUTC], F32, kind="ExternalInput")
    alpha11 = nc.dram_tensor("alpha11", [1, 1], F32, kind="ExternalInput")
    iota_in = nc.dram_tensor("iota_in", [P, P], BF16, kind="ExternalInput")
    ident_in = nc.dram_tensor("ident_in", [P, P], BF16, kind="ExternalInput")
    ones_row_in = nc.dram_tensor("ones_row_in", [1, P], F32, kind="ExternalInput")
    col_adj = nc.dram_tensor("col_adj", [P, TE], I32, kind="ExternalInput")
    dr_edge = nc.dram_tensor("dr_edge", [P, TE], I32, kind="ExternalInput")
    d_own = nc.dram_tensor("d_own", [P, NT_LOC], I32, kind="ExternalInput")

    out = nc.dram_tensor("out", [NPC, OUTC], F32, kind="ExternalOutput")

    with tile.TileContext(nc) as tc:
        with tc.tile_pool(name="const", bufs=1) as const:
            # phase-2-critical loads issue first on the sync queue
            Wq_t = const.tile([P, 2 * C], BF16)
            nc.sync.dma_start(out=Wq_t[:].rearrange("p (c n) -> p c n", c=2), in_=WqT[:].rearrange("(c p) n -> p c n", p=P))
            Wk_t = const.tile([P, 2 * C], BF16)
            nc.sync.dma_start(out=Wk_t[:].rearrange("p (c n) -> p c n", c=2), in_=WkT[:].rearrange("(c p) n -> p c n", p=P))
            Wv_t = const.tile([P, 2 * C], BF16)
            nc.sync.dma_start(out=Wv_t[:].rearrange("p (c n) -> p c n", c=2), in_=WvT[:].rearrange("(c p) n -> p c n", p=P))
            xo = [const.tile([P, NPC], BF16, tag=f"xo{c}", name=f"xo{c}") for c in range(2)]
            for c in range(2):
                nc.sync.dma_start(out=xo[c][:], in_=xT_own[c * P:(c + 1) * P, :])
            xts = [const.tile([P, N], BF16, name=f"xts{c}") for c in range(2)]
            for c in range(2):
                nc.sync.dma_start(out=xts[c][:], in_=xT[c * P:(c + 1) * P, :])
            bq_t = const.tile([P, 2], F32)
            nc.sync.dma_start(out=bq_t[:], in_=bq_pack[:])
            bk_t = const.tile([P, 2], F32)
            nc.sync.dma_start(out=bk_t[:], in_=bk_pack[:])
            bv_t = const.tile([P, C], F32)
            nc.sync.dma_start(out=bv_t[:], in_=bv_rep[:])

            # ---- persistent constants ----
            iota_t = const.tile([P, P], BF16)
            nc.sync.dma_start(out=iota_t[:], in_=iota_in[:])
            ident_t = const.tile([P, P], BF16)
            nc.sync.dma_start(out=ident_t[:], in_=ident_in[:])
            ones_row_t = const.tile([1, P], F32)
            nc.sync.dma_start(out=ones_row_t[:], in_=ones_row_in[:])
            col_t = const.tile([P, TE], I32)
            nc.sync.dma_start(out=col_t[:], in_=col_adj[:])
            colf_t = const.tile([P, TE], F32)
            nc.vector.tensor_copy(out=colf_t[:], in_=col_t[:])
            expb_col = const.tile([P, 1], F32)
            nc.vector.memset(expb_col[:], EXP_BIAS)
            eps_col = const.tile([P, 1], F32)
            nc.vector.memset(eps_col[:], 1e-5)
            w_col = const.tile([P, 1], F32)

            # guarded rsqrt of integer degrees: d=0 -> 0, else 1/sqrt(d)
            rs_row = const.tile([P, TE], F32)
            s_own = const.tile([P, NT_LOC], F32)

            with tc.tile_pool(name="ph1", bufs=1) as ph1, \
                 tc.tile_pool(name="ps1", bufs=1, space="PSUM") as ps1:
                al_t = ph1.tile([1, 1], F32)
                nc.sync.dma_start(out=al_t[:], in_=alpha11[:])
                wsig = ph1.tile([1, 1], F32)
                nc.scalar.activation(out=wsig[:], in_=al_t[:], func=AF.Sigmoid)
                wrep_ps = ps1.tile([P, 1], F32, tag="wrep")
                nc.tensor.matmul(out=wrep_ps[:], lhsT=ones_row_t[:], rhs=wsig[:],
                                 start=True, stop=True)
                nc.vector.tensor_copy(out=w_col[:], in_=wrep_ps[:])

                for (src_dram, dst, w_) in ((dr_edge, rs_row, TE), (d_own, s_own, NT_LOC)):
                    di = ph1.tile([P, w_], I32, tag=f"di{w_}")
                    nc.sync.dma_start(out=di[:], in_=src_dram[:])
                    df = ph1.tile([P, w_], F32, tag=f"df{w_}")
                    nc.vector.tensor_copy(out=df[:], in_=di[:])
                    m_t = ph1.tile([P, w_], F32, tag=f"m{w_}")
                    nc.vector.tensor_scalar(out=m_t[:], in0=df[:], scalar1=1.0,
                                            scalar2=None, op0=OP.min)
                    t1 = ph1.tile([P, w_], F32, tag=f"t1{w_}")
                    nc.vector.tensor_scalar(out=t1[:], in0=df[:], scalar1=1.0,
                                            scalar2=None, op0=OP.add)
                    nc.vector.tensor_tensor(out=t1[:], in0=t1[:], in1=m_t[:],
                                            op=OP.subtract)
                    nc.scalar.activation(out=t1[:], in_=t1[:], func=AF.Sqrt)
                    nc.vector.reciprocal(out=t1[:], in_=t1[:])
                    nc.vector.tensor_tensor(out=dst[:], in0=t1[:], in1=m_t[:],
                                            op=OP.mult)

            # ================= phase 2: QKV projections (bf16) =================
            big = const
            KTp = [big.tile([P, N], BF16, name=f"KT{p}") for p in range(2)]
            QTp = [big.tile([P, NPC], BF16, name=f"QT{p}") for p in range(2)]
            Vt = big.tile([P, NT_GLOB * H * (DH + 1)], BF16, name="Vt")
            V4 = Vt[:].rearrange("p (k h d) -> p k h d", h=H, d=DH + 1)
            OTu_sb = [big.tile([DH + 1, NPC], BF16, name=f"OTu{h}") for h in range(H)]
            hi_sb = [big.tile([P, C], BF16, name=f"hi{i}") for i in range(NT_LOC)]

            nc.vector.memset(V4[:, :, :, DH:DH + 1], 1.0)  # ones column for denominators

            with tc.tile_pool(name="ph2", bufs=1) as ph2, \
                 tc.tile_pool(name="ps2", bufs=1, space="PSUM") as ps2:
                for p in range(2):
                    qps = ps2.tile([P, NPC], F32, tag="qkps", bufs=2)
                    for c in range(2):
                        for nb in range(NPC // 512):
                            mi = nc.tensor.matmul(
                                out=qps[:, nb * 512:(nb + 1) * 512],
                                lhsT=Wq_t[:, c * C + p * P: c * C + (p + 1) * P],
                                rhs=xo[c][:, nb * 512:(nb + 1) * 512],
                                start=(c == 0), stop=(c == 1))
                            if nb > 0:
                                mi.ins.ldweights = False  # same weight chunk
                    nc.scalar.activation(
                        out=QTp[p][:], in_=qps[:],
                        func=AF.Identity, bias=bq_t[:, p:p + 1])

                # K and V in slabs of 1024 nodes from the two resident xT halves
                SLAB = 1024
                for s in range(N // SLAB):
                    for p in range(2):
                        kps = ps2.tile([P, SLAB], F32, tag="qkps", bufs=2)
                        for c in range(2):
                            for nb in range(SLAB // 512):
                                nc.tensor.matmul(
                                    out=kps[:, nb * 512:(nb + 1) * 512],
                                    lhsT=Wk_t[:, c * C + p * P: c * C + (p + 1) * P],
                                    rhs=xts[c][:, s * SLAB + nb * 512:s * SLAB + (nb + 1) * 512],
                                    start=(c == 0), stop=(c == 1))
                        nc.scalar.activation(
                            out=KTp[p][:, s * SLAB:(s + 1) * SLAB],
                            in_=kps[:], func=AF.Identity, bias=bk_t[:, p:p + 1])
                    for ntl in range(SLAB // P):
                        g = s * (SLAB // P) + ntl
                        vps = ps2.tile([P, C], F32, tag="vps", bufs=2)
                        for c in range(2):
                            nc.tensor.matmul(
                                out=vps[:],
                                lhsT=xts[c][:, g * P:(g + 1) * P],
                                rhs=Wv_t[:, c * C:(c + 1) * C],
                                start=(c == 0), stop=(c == 1))
                        nc.vector.tensor_tensor(
                            out=V4[:, g, :, 0:DH],
                            in0=vps[:].rearrange("p (h d) -> p h d", d=DH),
                            in1=bv_t[:].rearrange("p (h d) -> p h d", d=DH),
                            op=OP.add)

            # ========== phase 3: attention + interleaved GCN scatter ==========
            with tc.tile_pool(name="ph3", bufs=1) as ph3, \
                 tc.tile_pool(name="ps3", bufs=1, space="PSUM") as ps3:

                # GCN scatter jobs, interleaved across attention steps so the
                # sequential xe DMA streams overlap attention compute.  xe is
                # loaded XB tiles per DMA to keep the SP queue issue rate low.
                XB = 8
                scat_jobs = [(t, i) for t in range(NT_LOC) for i in range(TPT)]
                n_jobs = len(scat_jobs)
                n_steps = H * NT_GLOB
                emitted = 0
                hips_cur = {}
                xeb_cur = [None]

                built = 0
                oh_ring = {}

                def emit_scatter_builds(upto):
                    # one-hot builds (DVE) and xe loads run AHEAD of their
                    # consuming matmuls so the PE never waits on fresh data.
                    nonlocal built
                    while built < min(upto, n_jobs):
                        j = built
                        if j % XB == 0:
                            nb = min(XB, TE - j)
                            xeb_cur[0] = ph3.tile([P, XB, C], BF16, tag="xet", bufs=3,
                                                  name=f"xeb{j}")
                            nc.sync.dma_start(
                                out=xeb_cur[0][:, 0:nb, :],
                                in_=xe[j * P:(j + nb) * P, :].rearrange(
                                    "(i p) c -> p i c", p=P))
                        # weighted one-hot: (iota == col) * rsqrt(d_row)
                        oh = ph3.tile([P, P], BF16, tag="oh2", bufs=10, name=f"oh{j}")
                        nc.vector.tensor_scalar(
                            out=oh[:], in0=iota_t[:], scalar1=colf_t[:, j:j + 1],
                            scalar2=rs_row[:, j:j + 1], op0=OP.is_equal, op1=OP.mult)
                        oh_ring[j] = (oh, xeb_cur[0])
                        built += 1

                def emit_scatter_jobs(upto):
                    nonlocal emitted
                    emit_scatter_builds(upto + 6)
                    while emitted < min(upto, n_jobs):
                        t, i = scat_jobs[emitted]
                        j = t * TPT + i
                        if i == 0:
                            hips_cur[t] = ps3.tile([P, C], F32, tag="hips", bufs=2, name=f"hips{t}")
                        oh, xeb = oh_ring.pop(j)
                        nc.tensor.matmul(out=hips_cur[t][:], lhsT=oh[:],
                                         rhs=xeb[:, j % XB, :],
                                         start=(i == 0), stop=(i == TPT - 1))
                        if i == TPT - 1:
                            nc.vector.tensor_scalar(out=hi_sb[t][:], in0=hips_cur[t][:],
                                                    scalar1=s_own[:, t:t + 1],
                                                    scalar2=None, op0=OP.mult)
                        emitted += 1

                for h in range(H):
                    p, hh = h // 2, h % 2
                    po = hh * DH
                    OT_ps = ps3.tile([DH + 1, NPC], F32, tag="OT", bufs=1, name=f"OTps{h}")

                    def emit_attnv(kt, et):
                        for qh in range(2):
                            mi = nc.tensor.matmul(
                                out=OT_ps[:, qh * 512:(qh + 1) * 512],
                                lhsT=V4[:, kt, h, :],
                                rhs=et[:, qh * 512:(qh + 1) * 512],
                                start=(kt == 0), stop=(kt == NT_GLOB - 1))
                            if qh == 1:
                                mi.ins.ldweights = False  # same V tile as qh=0

                    # software-pipelined: attn@V runs two steps behind exp, so
                    # the PE consumes exp outputs produced ~2.5 us earlier and
                    # never waits on semaphore propagation from the ACT engine.
                    DEPTH = 2
                    et_hist = []
                    for kt in range(NT_GLOB):
                        sps = ps3.tile([P, NPC], F32, tag="sps", bufs=2)
                        for qh in range(2):
                            nc.tensor.matmul(
                                out=sps[:, qh * 512:(qh + 1) * 512],
                                lhsT=KTp[p][po:po + DH, kt * P:(kt + 1) * P],
                                rhs=QTp[p][po:po + DH, qh * 512:(qh + 1) * 512],
                                start=True, stop=True)
                        et = ph3.tile([P, NPC], BF16, tag="expT", bufs=DEPTH + 2)
                        nc.scalar.activation(out=et[:], in_=sps[:], func=AF.Exp,
                                             bias=expb_col[:, 0:1], scale=1.0 / np.sqrt(DH))
                        et_hist.append(et)
                        step = h * NT_GLOB + kt + 1
                        emit_scatter_jobs(n_jobs * step // n_steps)
                        if kt >= DEPTH:
                            emit_attnv(kt - DEPTH, et_hist[kt - DEPTH])
                    for kt in range(NT_GLOB - DEPTH, NT_GLOB):
                        emit_attnv(kt, et_hist[kt])

                    # drain PSUM (releases the accumulator); normalization by
                    # the denominator happens at the start of phase 4.
                    nc.vector.tensor_copy(out=OTu_sb[h][:], in_=OT_ps[:])

            # ================= phase 4: out_proj, LN, combine, fc =================
            with tc.tile_pool(name="ph4", bufs=1) as ph4:
                Wop_t = ph4.tile([DH, H * C], BF16)
                nc.sync.dma_start(out=Wop_t[:], in_=Wop4[:])
                Wl_t = ph4.tile([P, 2 * C], BF16)
                nc.sync.dma_start(out=Wl_t[:].rearrange("p (c n) -> p c n", c=2), in_=Wl[:].rearrange("(c p) n -> p c n", p=P))
                fc_t = ph4.tile([P, 2 * OUTC], BF16)
                nc.sync.dma_start(out=fc_t[:].rearrange("p (c n) -> p c n", c=2), in_=fcT[:].rearrange("(c p) n -> p c n", p=P))
                opb_t = ph4.tile([P, C], F32)
                nc.sync.dma_start(out=opb_t[:], in_=opb_rep[:])
                g_t = ph4.tile([P, C], F32)
                nc.sync.dma_start(out=g_t[:], in_=g_rep[:])
                b_t = ph4.tile([P, C], F32)
                nc.sync.dma_start(out=b_t[:], in_=b_rep[:])
                fcb_t = ph4.tile([P, OUTC], F32)
                nc.sync.dma_start(out=fcb_t[:], in_=fcb_rep[:])

                # ---- softmax denominators: transpose to node-major via tiny
                # ones-matmuls, single cheap reciprocal; 1/den is applied
                # per-query inside the qt loop below.
                with tc.tile_pool(name="ps4a", bufs=1, space="PSUM") as ps4a:
                    on64 = ph4.tile([P, 1], BF16)
                    nc.vector.memset(on64[:], 1.0)
                    den_cols = ph4.tile([P, H * NT_LOC], F32)
                    for qt in range(NT_LOC):
                        for h in range(H):
                            tpc = ps4a.tile([P, 1], F32, tag="tpc", bufs=2)
                            nc.tensor.matmul(
                                out=tpc[:],
                                lhsT=OTu_sb[h][DH:DH + 1, qt * P:(qt + 1) * P],
                                rhs=on64[64:65, 0:1],
                                start=True, stop=True)
                            nc.vector.tensor_copy(
                                out=den_cols[:, qt * H + h:qt * H + h + 1], in_=tpc[:])
                    rden_cols = ph4.tile([P, H * NT_LOC], F32)
                    nc.vector.reciprocal(out=rden_cols[:], in_=den_cols[:])

                ps4 = tc.alloc_tile_pool(name="ps4", bufs=1, space="PSUM")

                def transpose_2chunks(src_ap, tag, on_act=False):
                    dst = ph4.tile([P, C], BF16, tag=tag, bufs=2, name=f"t2c{tag}")
                    for c in range(2):
                        tp = ps4.tile([P, P], BF16, tag="tp", bufs=2)
                        nc.tensor.transpose(out=tp[:], in_=src_ap[:, c * P:(c + 1) * P],
                                            identity=ident_t[:])
                        if on_act:
                            nc.scalar.copy(out=dst[:, c * P:(c + 1) * P], in_=tp[:])
                        else:
                            nc.vector.tensor_copy(out=dst[:, c * P:(c + 1) * P], in_=tp[:])
                    return dst

                for qt in range(NT_LOC):
                    # ---- global path: per-head out_proj, scaled by 1/den per
                    # query while summing heads (ACT/DVE split) ----
                    xo_t = ph4.tile([P, C], F32, tag="xot", bufs=2)
                    nc.sync.dma_start(out=xo_t[:], in_=x_own[qt * P:(qt + 1) * P, :])
                    cps = [ps4.tile([P, C], F32, tag="cps", bufs=4, name=f"cps{h}_{qt}")
                           for h in range(H)]
                    for h in range(H):
                        nc.tensor.matmul(
                            out=cps[h][:],
                            lhsT=OTu_sb[h][0:DH, qt * P:(qt + 1) * P],
                            rhs=Wop_t[:, h * C:(h + 1) * C],
                            start=True, stop=True)
                    vs = [ph4.tile([P, C], F32, tag="vs", bufs=8, name=f"vs{h}_{qt}")
                          for h in range(H)]
                    for h in range(H):
                        r_ap = rden_cols[:, qt * H + h:qt * H + h + 1]
                        if h % 2 == 0:
                            nc.scalar.activation(out=vs[h][:], in_=cps[h][:],
                                                 func=AF.Copy, scale=r_ap)
                        else:
                            nc.vector.tensor_scalar(out=vs[h][:], in0=cps[h][:],
                                                    scalar1=r_ap, scalar2=None,
                                                    op0=OP.mult)
                    xob = ph4.tile([P, C], F32, tag="xob", bufs=2)
                    nc.vector.tensor_tensor(out=xob[:], in0=xo_t[:], in1=opb_t[:],
                                            op=OP.add)
                    nc.vector.tensor_tensor(out=vs[0][:], in0=vs[0][:], in1=vs[1][:],
                                            op=OP.add)
                    nc.gpsimd.tensor_tensor(out=vs[2][:], in0=vs[2][:], in1=vs[3][:],
                                            op=OP.add)
                    nc.vector.tensor_tensor(out=vs[0][:], in0=vs[0][:], in1=vs[2][:],
                                            op=OP.add)
                    v_t = ph4.tile([P, C], F32, tag="vt", bufs=2)
                    nc.vector.tensor_tensor(out=v_t[:], in0=vs[0][:], in1=xob[:],
                                            op=OP.add)
                    msum = ph4.tile([P, 1], F32, tag="msum", bufs=2)
                    nc.vector.reduce_sum(out=msum[:], in_=v_t[:], axis=AX.X)
                    mean_neg = ph4.tile([P, 1], F32, tag="mean", bufs=2)
                    nc.vector.tensor_scalar(out=mean_neg[:], in0=msum[:],
                                            scalar1=-1.0 / C, scalar2=None, op0=OP.mult)
                    nc.scalar.activation(out=v_t[:], in_=v_t[:], func=AF.Identity,
                                         bias=mean_neg[:, 0:1])
                    sq = ph4.tile([P, C], F32, tag="sq", bufs=2)
                    ssum = ph4.tile([P, 1], F32, tag="ssum", bufs=2)
                    nc.scalar.activation(out=sq[:], in_=v_t[:], func=AF.Square,
                                         accum_out=ssum[:])
                    sstd = ph4.tile([P, 1], F32, tag="sstd", bufs=2)
                    nc.scalar.activation(out=sstd[:], in_=ssum[:], func=AF.Sqrt,
                                         bias=eps_col[:, 0:1], scale=1.0 / C)
                    rstd = ph4.tile([P, 1], F32, tag="rstd", bufs=2)
                    nc.vector.reciprocal(out=rstd[:], in_=sstd[:])
                    nc.scalar.activation(out=v_t[:], in_=v_t[:], func=AF.Copy,
                                         scale=rstd[:, 0:1])
                    nc.vector.tensor_tensor(out=v_t[:], in0=v_t[:], in1=g_t[:], op=OP.mult)
                    nc.vector.tensor_tensor(out=v_t[:], in0=v_t[:], in1=b_t[:], op=OP.add)

                    # ---- local path: local_embed = hi @ W_local ----
                    hiT = transpose_2chunks(hi_sb[qt][:], "hiT", on_act=True)
                    lps = ps4.tile([P, C], F32, tag="lps", bufs=2)
                    for c in range(2):
                        nc.tensor.matmul(out=lps[:], lhsT=hiT[:, c * P:(c + 1) * P],
                                         rhs=Wl_t[:, c * C:(c + 1) * C],
                                         start=(c == 0), stop=(c == 1))
                    # combined = global + w * (local - global)
                    comb = ph4.tile([P, C], F32, tag="comb", bufs=2)
                    nc.vector.tensor_tensor(out=comb[:], in0=lps[:], in1=v_t[:],
                                            op=OP.subtract)
                    nc.scalar.activation(out=comb[:], in_=comb[:], func=AF.Copy,
                                         scale=w_col[:, 0:1])
                    nc.vector.tensor_tensor(out=comb[:], in0=comb[:], in1=v_t[:], op=OP.add)
                    combb = ph4.tile([P, C], BF16, tag="combb", bufs=2)
                    nc.scalar.copy(out=combb[:], in_=comb[:])

                    # ---- fc ----
                    cT = transpose_2chunks(combb[:], "cT", on_act=True)
                    fps = ps4.tile([P, OUTC], F32, tag="lps", bufs=2, name=f"fps{qt}")
                    for c in range(2):
                        nc.tensor.matmul(out=fps[:], lhsT=cT[:, c * P:(c + 1) * P],
                                         rhs=fc_t[:, c * OUTC:(c + 1) * OUTC],
                                         start=(c == 0), stop=(c == 1))
                    o_t = ph4.tile([P, OUTC], F32, tag="ot", bufs=2)
                    nc.vector.tensor_tensor(out=o_t[:], in0=fps[:], in1=fcb_t[:], op=OP.add)
                    nc.sync.dma_start(out=out[qt * P:(qt + 1) * P, :], in_=o_t[:])
                ps4.release()
    nc.finalize()
    return nc


def _prep_edges(adj):
    """Bucket edges by destination node-tile; pad segments to a common length.

    Pure relayout/counting on the host: per-core local col indices, per-edge
    source-row indices (used to gather x into edge order), integer degrees.
    """
    row = np.asarray(adj[0], dtype=np.int64)
    col = np.asarray(adj[1], dtype=np.int64)
    d = np.bincount(col, minlength=N).astype(np.int32)
    tid = col // P
    order = np.argsort(tid, kind='stable')
    row_s, col_s = row[order], col[order]
    counts = np.bincount(tid, minlength=NT_GLOB)
    S = int(np.ceil(max(counts.max(), 1) / P) * P)
    TPT = S // P
    col_pad = np.full((NT_GLOB, S), -1, dtype=np.int32)
    row_pad = np.zeros((NT_GLOB, S), dtype=np.int32)
    start = 0
    for g in range(NT_GLOB):
        cnt = int(counts[g])
        col_pad[g, :cnt] = (col_s[start:start + cnt] - g * P).astype(np.int32)
        row_pad[g, :cnt] = row_s[start:start + cnt].astype(np.int32)
        start += cnt
    TE = NT_LOC * TPT
    per_core = []
    for k in range(NCORES):
        cols_k = col_pad[NT_LOC * k:NT_LOC * (k + 1)].reshape(TE, P)
        rows_k = row_pad[NT_LOC * k:NT_LOC * (k + 1)].reshape(TE, P)
        ca = np.ascontiguousarray(cols_k.T)                    # [P, TE]
        dre = np.ascontiguousarray(d[rows_k].T)                # [P, TE]
        down = np.ascontiguousarray(
            d[k * NPC:(k + 1) * NPC].reshape(NT_LOC, P).T)     # [P, NT_LOC]
        per_core.append((ca, rows_k.reshape(-1), dre, down))
    return per_core, TPT


def kernel(x, adj, weight_local, in_proj_w, in_proj_b, out_proj_w, out_proj_b,
           ln_g, ln_b, alpha, fc_w, fc_b):
    global LAST_RESULTS
    x = np.ascontiguousarray(np.asarray(x, dtype=np.float32))
    per_core_edges, TPT = _prep_edges(np.asarray(adj))

    bf = ml_dtypes.bfloat16
    x_bf = x.astype(bf)
    xT_bf = np.ascontiguousarray(x_bf.T)
    WopT = np.asarray(out_proj_w).T.astype(np.float32)  # [C_in, C_out]
    Wop4 = np.ascontiguousarray(
        WopT.reshape(H, DH, C).transpose(1, 0, 2).reshape(DH, H * C).astype(bf))
    common = dict(
        xT=xT_bf,
        WqT=np.ascontiguousarray(np.asarray(in_proj_w)[0:C].T.astype(bf)),
        WkT=np.ascontiguousarray(np.asarray(in_proj_w)[C:2 * C].T.astype(bf)),
        WvT=np.ascontiguousarray(np.asarray(in_proj_w)[2 * C:3 * C].T.astype(bf)),
        Wop4=Wop4,
        Wl=np.ascontiguousarray(np.asarray(weight_local).astype(bf)),
        fcT=np.ascontiguousarray(np.asarray(fc_w).T.astype(bf)),
        bq_pack=np.ascontiguousarray(np.asarray(in_proj_b)[0:C].astype(np.float32).reshape(2, P).T),
        bk_pack=np.ascontiguousarray(np.asarray(in_proj_b)[C:2 * C].astype(np.float32).reshape(2, P).T),
        bv_rep=np.tile(np.asarray(in_proj_b)[2 * C:3 * C].astype(np.float32), (P, 1)),
        opb_rep=np.tile(np.asarray(out_proj_b, dtype=np.float32), (P, 1)),
        g_rep=np.tile(np.asarray(ln_g, dtype=np.float32), (P, 1)),
        b_rep=np.tile(np.asarray(ln_b, dtype=np.float32), (P, 1)),
        fcb_rep=np.tile(np.asarray(fc_b, dtype=np.float32), (P, 1)),
        alpha11=np.asarray(alpha, dtype=np.float32).reshape(1, 1),
        iota_in=np.tile(np.arange(P, dtype=np.float32), (P, 1)).astype(bf),
        ident_in=np.eye(P, dtype=np.float32).astype(bf),
        ones_row_in=np.ones((1, P), dtype=np.float32),
    )
    in_maps = []
    for k in range(NCORES):
        ca, rows_flat, dre, down = per_core_edges[k]
        m = dict(common)
        m['xT_own'] = np.ascontiguousarray(xT_bf[:, k * NPC:(k + 1) * NPC])
        m['x_own'] = np.ascontiguousarray(x[k * NPC:(k + 1) * NPC, :])
        m['xe'] = np.ascontiguousarray(x_bf[rows_flat])
        m['col_adj'] = ca
        m['dr_edge'] = dre
        m['d_own'] = down
        in_maps.append(m)

    nc = _build(TPT)
    res = run_bass_kernel_spmd(nc, in_maps, core_ids=list(range(NCORES)))
    LAST_RESULTS = res
    return np.concatenate([res.results[k]['out'] for k in range(NCORES)], axis=0)


# revision 48
# speedup vs baseline: 1.0240x; 1.0058x over previous
"""Trainium2 Bass kernel for LocalGlobalEnvEncoder (GCN + MHA fusion).

Sharding: nodes are split across the 8 cores (1024 dest nodes / queries each).
 - GCN: edges bucketed by destination node-tile on host (layout only); source
   features are laid out in edge order on host (a pure gather / relayout), so
   the device streams them with plain sequential DMA instead of per-row
   indirect gathers. Per-edge 1/sqrt(d_row) weights are folded into the
   one-hot scatter matrix (built on DVE with a fused is_equal*mult chain) and
   the scatter-add runs on the PE in bf16.  Degrees are integer counts
   (host-side bincount relayout); all floating-point math (rsqrt, scaling,
   matmuls) happens on device.
 - MHA: query-sharded attention, K/V computed redundantly per core in bf16.
   Scores are kept transposed ([key, query]); exp runs on the ACT engine
   writing bf16; attn@V uses V as the stationary operand accumulating
   O^T [head_dim+1, queries] over key tiles, with softmax denominators coming
   from an appended ones-column in V.  Normalization (1/den) is applied
   per-query in node-major layout during the output projection.
All floating-point math happens on device; the host only re-lays-out inputs.
"""
import sys
sys.path.insert(0, '/opt/trn_rl_repo')
import numpy as np
import ml_dtypes
import concourse.bass as bass
import concourse.tile as tile
from concourse import bacc, mybir
from concourse.bass_utils import run_bass_kernel_spmd

F32 = mybir.dt.float32
BF16 = mybir.dt.bfloat16
I32 = mybir.dt.int32
AF = mybir.ActivationFunctionType
OP = mybir.AluOpType
AX = mybir.AxisListType

N, E, C, OUTC, H, DH = 8192, 262144, 256, 256, 4, 64
NCORES = 8
NPC = N // NCORES          # nodes per core = 1024
P = 128
NT_LOC = NPC // P          # node tiles per core = 8
NT_GLOB = N // P           # global node tiles = 64
EXP_BIAS = -12.0           # uniform shift inside softmax exp; cancels in the ratio

LAST_RESULTS = None        # stashed BassKernelResults for test harness introspection


def _build(TPT):
    """Build the single SPMD Bass program. TPT = edge tiles per node-tile segment."""
    nc = bacc.Bacc('TRN2', target_bir_lowering=False, debug=False, num_devices=NCORES)
    TE = NT_LOC * TPT  # total edge tiles per core

    # ---- I/O ----
    xT = nc.dram_tensor("xT", [C, N], BF16, kind="ExternalInput")
    xT_own = nc.dram_tensor("xT_own", [C, NPC], BF16, kind="ExternalInput")
    x_own = nc.dram_tensor("x_own", [NPC, C], F32, kind="ExternalInput")
    xe = nc.dram_tensor("xe", [TE * P, C], BF16, kind="ExternalInput")
    WqT = nc.dram_tensor("WqT", [C, C], BF16, kind="ExternalInput")
    WkT = nc.dram_tensor("WkT", [C, C], BF16, kind="ExternalInput")
    WvT = nc.dram_tensor("WvT", [C, C], BF16, kind="ExternalInput")
    Wop4 = nc.dram_tensor("Wop4", [DH, H * C], BF16, kind="ExternalInput")
    Wl = nc.dram_tensor("Wl", [C, C], BF16, kind="ExternalInput")
    fcT = nc.dram_tensor("fcT", [C, OUTC], BF16, kind="ExternalInput")
    bq_pack = nc.dram_tensor("bq_pack", [P, 2], F32, kind="ExternalInput")
    bk_pack = nc.dram_tensor("bk_pack", [P, 2], F32, kind="ExternalInput")
    bv_rep = nc.dram_tensor("bv_rep", [P, C], F32, kind="ExternalInput")
    opb_rep = nc.dram_tensor("opb_rep", [P, C], F32, kind="ExternalInput")
    g_rep = nc.dram_tensor("g_rep", [P, C], F32, kind="ExternalInput")
    b_rep = nc.dram_tensor("b_rep", [P, C], F32, kind="ExternalInput")
    fcb_rep = nc.dram_tensor("fcb_rep", [P, OUTC], F32, kind="ExternalInput")
    alpha11 = nc.dram_tensor("alpha11", [1, 1], F32, kind="ExternalInput")
    iota_in = nc.dram_tensor("iota_in", [P, P], BF16, kind="ExternalInput")
    ident_in = nc.dram_tensor("ident_in", [P, P], BF16, kind="ExternalInput")
    ident32_in = nc.dram_tensor("ident32_in", [P, P], F32, kind="ExternalInput")
    ones_row_in = nc.dram_tensor("ones_row_in", [1, P], F32, kind="ExternalInput")
    col_adj = nc.dram_tensor("col_adj", [P, TE], I32, kind="ExternalInput")
    dr_edge = nc.dram_tensor("dr_edge", [P, TE], I32, kind="ExternalInput")
    d_own = nc.dram_tensor("d_own", [P, NT_LOC], I32, kind="ExternalInput")

    out = nc.dram_tensor("out", [NPC, OUTC], F32, kind="ExternalOutput")

    with tile.TileContext(nc) as tc:
        with tc.tile_pool(name="const", bufs=1) as const:
            # phase-2-critical loads issue first on the sync queue
            Wq_t = const.tile([P, 2 * C], BF16)
            nc.sync.dma_start(out=Wq_t[:].rearrange("p (c n) -> p c n", c=2), in_=WqT[:].rearrange("(c p) n -> p c n", p=P))
            Wk_t = const.tile([P, 2 * C], BF16)
            nc.sync.dma_start(out=Wk_t[:].rearrange("p (c n) -> p c n", c=2), in_=WkT[:].rearrange("(c p) n -> p c n", p=P))
            Wv_t = const.tile([P, 2 * C], BF16)
            nc.sync.dma_start(out=Wv_t[:].rearrange("p (c n) -> p c n", c=2), in_=WvT[:].rearrange("(c p) n -> p c n", p=P))
            xo = [const.tile([P, NPC], BF16, tag=f"xo{c}", name=f"xo{c}") for c in range(2)]
            for c in range(2):
                nc.sync.dma_start(out=xo[c][:], in_=xT_own[c * P:(c + 1) * P, :])
            xts = [const.tile([P, N], BF16, name=f"xts{c}") for c in range(2)]
            for c in range(2):
                nc.sync.dma_start(out=xts[c][:, 0:N // 2],
                                  in_=xT[c * P:(c + 1) * P, 0:N // 2])
            for c in range(2):
                nc.sync.dma_start(out=xts[c][:, N // 2:N],
                                  in_=xT[c * P:(c + 1) * P, N // 2:N])
            bq_t = const.tile([P, 2], F32)
            nc.sync.dma_start(out=bq_t[:], in_=bq_pack[:])
            bk_t = const.tile([P, 2], F32)
            nc.sync.dma_start(out=bk_t[:], in_=bk_pack[:])
            bv_t = const.tile([P, C], F32)
            nc.sync.dma_start(out=bv_t[:], in_=bv_rep[:])

            # ---- persistent constants ----
            iota_t = const.tile([P, P], BF16)
            nc.sync.dma_start(out=iota_t[:], in_=iota_in[:])
            ident_t = const.tile([P, P], BF16)
            nc.sync.dma_start(out=ident_t[:], in_=ident_in[:])
            ones_row_t = const.tile([1, P], F32)
            nc.sync.dma_start(out=ones_row_t[:], in_=ones_row_in[:])
            col_t = const.tile([P, TE], I32)
            nc.sync.dma_start(out=col_t[:], in_=col_adj[:])
            colf_t = const.tile([P, TE], F32)
            nc.vector.tensor_copy(out=colf_t[:], in_=col_t[:])
            expb_col = const.tile([P, 1], F32)
            nc.vector.memset(expb_col[:], EXP_BIAS)
            eps_col = const.tile([P, 1], F32)
            nc.vector.memset(eps_col[:], 1e-5)
            w_col = const.tile([P, 1], F32)
            w1m_col = const.tile([P, 1], F32)

            # guarded rsqrt of integer degrees: d=0 -> 0, else 1/sqrt(d)
            rs_row = const.tile([P, TE], F32)
            s_own = const.tile([P, NT_LOC], F32)

            with tc.tile_pool(name="ph1", bufs=1) as ph1, \
                 tc.tile_pool(name="ps1", bufs=1, space="PSUM") as ps1:
                al_t = ph1.tile([1, 1], F32)
                nc.sync.dma_start(out=al_t[:], in_=alpha11[:])
                wsig = ph1.tile([1, 1], F32)
                nc.scalar.activation(out=wsig[:], in_=al_t[:], func=AF.Sigmoid)
                wrep_ps = ps1.tile([P, 1], F32, tag="wrep")
                nc.tensor.matmul(out=wrep_ps[:], lhsT=ones_row_t[:], rhs=wsig[:],
                                 start=True, stop=True)
                nc.vector.tensor_copy(out=w_col[:], in_=wrep_ps[:])
                nc.vector.tensor_scalar(out=w1m_col[:], in0=w_col[:], scalar1=-1.0,
                                        scalar2=1.0, op0=OP.mult, op1=OP.add)

                for (src_dram, dst, w_) in ((dr_edge, rs_row, TE), (d_own, s_own, NT_LOC)):
                    di = ph1.tile([P, w_], I32, tag=f"di{w_}")
                    nc.sync.dma_start(out=di[:], in_=src_dram[:])
                    df = ph1.tile([P, w_], F32, tag=f"df{w_}")
                    nc.vector.tensor_copy(out=df[:], in_=di[:])
                    m_t = ph1.tile([P, w_], F32, tag=f"m{w_}")
                    nc.vector.tensor_scalar(out=m_t[:], in0=df[:], scalar1=1.0,
                                            scalar2=None, op0=OP.min)
                    t1 = ph1.tile([P, w_], F32, tag=f"t1{w_}")
                    nc.vector.tensor_scalar(out=t1[:], in0=df[:], scalar1=1.0,
                                            scalar2=None, op0=OP.add)
                    nc.vector.tensor_tensor(out=t1[:], in0=t1[:], in1=m_t[:],
                                            op=OP.subtract)
                    nc.scalar.activation(out=t1[:], in_=t1[:], func=AF.Sqrt)
                    nc.vector.reciprocal(out=t1[:], in_=t1[:])
                    nc.vector.tensor_tensor(out=dst[:], in0=t1[:], in1=m_t[:],
                                            op=OP.mult)

            # ================= phase 2: QKV projections (bf16) =================
            big = const
            KTp = [big.tile([P, N], BF16, name=f"KT{p}") for p in range(2)]
            QTp = [big.tile([P, NPC], BF16, name=f"QT{p}") for p in range(2)]
            Vt = big.tile([P, NT_GLOB * H * (DH + 1)], BF16, name="Vt")
            V4 = Vt[:].rearrange("p (k h d) -> p k h d", h=H, d=DH + 1)
            OTu_sb = [big.tile([DH + 1, NPC], BF16, name=f"OTu{h}") for h in range(H)]
            hi_sb = [big.tile([P, C], F32, name=f"hi{i}") for i in range(NT_LOC)]

            nc.vector.memset(V4[:, :, :, DH:DH + 1], 1.0)  # ones column for denominators

            with tc.tile_pool(name="ph2", bufs=1) as ph2, \
                 tc.tile_pool(name="ps2", bufs=1, space="PSUM") as ps2:
                for p in range(2):
                    qps = ps2.tile([P, NPC], F32, tag="qkps", bufs=2)
                    for c in range(2):
                        for nb in range(NPC // 512):
                            mi = nc.tensor.matmul(
                                out=qps[:, nb * 512:(nb + 1) * 512],
                                lhsT=Wq_t[:, c * C + p * P: c * C + (p + 1) * P],
                                rhs=xo[c][:, nb * 512:(nb + 1) * 512],
                                start=(c == 0), stop=(c == 1))
                            if nb > 0:
                                mi.ins.ldweights = False  # same weight chunk
                    nc.scalar.activation(
                        out=QTp[p][:], in_=qps[:],
                        func=AF.Identity, bias=bq_t[:, p:p + 1])

                # K and V in slabs of 1024 nodes from the two resident xT halves
                SLAB = 1024
                for s in range(N // SLAB):
                    for p in range(2):
                        kps = ps2.tile([P, SLAB], F32, tag="qkps", bufs=2)
                        for c in range(2):
                            for nb in range(SLAB // 512):
                                nc.tensor.matmul(
                                    out=kps[:, nb * 512:(nb + 1) * 512],
                                    lhsT=Wk_t[:, c * C + p * P: c * C + (p + 1) * P],
                                    rhs=xts[c][:, s * SLAB + nb * 512:s * SLAB + (nb + 1) * 512],
                                    start=(c == 0), stop=(c == 1))
                        nc.scalar.activation(
                            out=KTp[p][:, s * SLAB:(s + 1) * SLAB],
                            in_=kps[:], func=AF.Identity, bias=bk_t[:, p:p + 1])
                    for ntl in range(SLAB // P):
                        g = s * (SLAB // P) + ntl
                        vps = ps2.tile([P, C], F32, tag="vps", bufs=2)
                        for c in range(2):
                            nc.tensor.matmul(
                                out=vps[:],
                                lhsT=xts[c][:, g * P:(g + 1) * P],
                                rhs=Wv_t[:, c * C:(c + 1) * C],
                                start=(c == 0), stop=(c == 1))
                        nc.vector.tensor_tensor(
                            out=V4[:, g, :, 0:DH],
                            in0=vps[:].rearrange("p (h d) -> p h d", d=DH),
                            in1=bv_t[:].rearrange("p (h d) -> p h d", d=DH),
                            op=OP.add)

            # ========== phase 3: attention + interleaved GCN scatter ==========
            with tc.tile_pool(name="ph3", bufs=1) as ph3, \
                 tc.tile_pool(name="ps3", bufs=1, space="PSUM") as ps3:

                # GCN scatter jobs, interleaved across attention steps so the
                # sequential xe DMA streams overlap attention compute.  xe is
                # loaded XB tiles per DMA to keep the SP queue issue rate low.
                XB = 8
                scat_jobs = [(t, i) for t in range(NT_LOC) for i in range(TPT)]
                n_jobs = len(scat_jobs)
                n_steps = H * NT_GLOB
                emitted = 0
                hips_cur = {}
                xeb_cur = [None]

                built = 0
                oh_ring = {}

                def emit_scatter_builds(upto):
                    # one-hot builds (DVE) and xe loads run AHEAD of their
                    # consuming matmuls so the PE never waits on fresh data.
                    nonlocal built
                    while built < min(upto, n_jobs):
                        j = built
                        if j % XB == 0:
                            nb = min(XB, TE - j)
                            xeb_cur[0] = ph3.tile([P, XB, C], BF16, tag="xet", bufs=3,
                                                  name=f"xeb{j}")
                            nc.sync.dma_start(
                                out=xeb_cur[0][:, 0:nb, :],
                                in_=xe[j * P:(j + nb) * P, :].rearrange(
                                    "(i p) c -> p i c", p=P))
                        # weighted one-hot: (iota == col) * rsqrt(d_row)
                        oh = ph3.tile([P, P], BF16, tag="oh2", bufs=10, name=f"oh{j}")
                        nc.vector.tensor_scalar(
                            out=oh[:], in0=iota_t[:], scalar1=colf_t[:, j:j + 1],
                            scalar2=rs_row[:, j:j + 1], op0=OP.is_equal, op1=OP.mult)
                        oh_ring[j] = (oh, xeb_cur[0])
                        built += 1

                def emit_scatter_jobs(upto):
                    nonlocal emitted
                    emit_scatter_builds(upto + 6)
                    while emitted < min(upto, n_jobs):
                        t, i = scat_jobs[emitted]
                        j = t * TPT + i
                        if i == 0:
                            hips_cur[t] = ps3.tile([P, C], F32, tag="hips", bufs=2, name=f"hips{t}")
                        oh, xeb = oh_ring.pop(j)
                        nc.tensor.matmul(out=hips_cur[t][:], lhsT=oh[:],
                                         rhs=xeb[:, j % XB, :],
                                         start=(i == 0), stop=(i == TPT - 1))
                        if i == TPT - 1:
                            nc.vector.tensor_scalar(out=hi_sb[t][:], in0=hips_cur[t][:],
                                                    scalar1=s_own[:, t:t + 1],
                                                    scalar2=None, op0=OP.mult)
                        emitted += 1

                for h in range(H):
                    p, hh = h // 2, h % 2
                    po = hh * DH
                    OT_ps = ps3.tile([DH + 1, NPC], F32, tag="OT", bufs=1, name=f"OTps{h}")

                    def emit_attnv(kt, et):
                        for qh in range(2):
                            mi = nc.tensor.matmul(
                                out=OT_ps[:, qh * 512:(qh + 1) * 512],
                                lhsT=V4[:, kt, h, :],
                                rhs=et[:, qh * 512:(qh + 1) * 512],
                                start=(kt == 0), stop=(kt == NT_GLOB - 1))
                            if qh == 1:
                                mi.ins.ldweights = False  # same V tile as qh=0

                    # software-pipelined: attn@V runs two steps behind exp, so
                    # the PE consumes exp outputs produced ~2.5 us earlier and
                    # never waits on semaphore propagation from the ACT engine.
                    DEPTH = 2
                    et_hist = []
                    for kt in range(NT_GLOB):
                        sps = ps3.tile([P, NPC], F32, tag="sps", bufs=2)
                        for qh in range(2):
                            nc.tensor.matmul(
                                out=sps[:, qh * 512:(qh + 1) * 512],
                                lhsT=KTp[p][po:po + DH, kt * P:(kt + 1) * P],
                                rhs=QTp[p][po:po + DH, qh * 512:(qh + 1) * 512],
                                start=True, stop=True)
                        et = ph3.tile([P, NPC], BF16, tag="expT", bufs=DEPTH + 2)
                        nc.scalar.activation(out=et[:], in_=sps[:], func=AF.Exp,
                                             bias=expb_col[:, 0:1], scale=1.0 / np.sqrt(DH))
                        et_hist.append(et)
                        step = h * NT_GLOB + kt + 1
                        emit_scatter_jobs(n_jobs * step // n_steps)
                        if kt >= DEPTH:
                            emit_attnv(kt - DEPTH, et_hist[kt - DEPTH])
                    for kt in range(NT_GLOB - DEPTH, NT_GLOB):
                        emit_attnv(kt, et_hist[kt])

                    # drain PSUM (releases the accumulator); normalization by
                    # the denominator happens at the start of phase 4.
                    nc.vector.tensor_copy(out=OTu_sb[h][:], in_=OT_ps[:])

            # ================= phase 4: out_proj, LN, combine, fc =================
            with tc.tile_pool(name="ph4", bufs=1) as ph4:
                Wop_t = ph4.tile([DH, H * C], BF16)
                nc.sync.dma_start(out=Wop_t[:], in_=Wop4[:])
                Wl_t = ph4.tile([P, 2 * C], BF16)
                nc.sync.dma_start(out=Wl_t[:].rearrange("p (c n) -> p c n", c=2), in_=Wl[:].rearrange("(c p) n -> p c n", p=P))
                fc_t = ph4.tile([P, 2 * OUTC], BF16)
                nc.sync.dma_start(out=fc_t[:].rearrange("p (c n) -> p c n", c=2), in_=fcT[:].rearrange("(c p) n -> p c n", p=P))
                opb_t = ph4.tile([P, C], F32)
                nc.sync.dma_start(out=opb_t[:], in_=opb_rep[:])
                g_t = ph4.tile([P, C], F32)
                nc.sync.dma_start(out=g_t[:], in_=g_rep[:])
                b_t = ph4.tile([P, C], F32)
                nc.sync.dma_start(out=b_t[:], in_=b_rep[:])
                identf = ph4.tile([P, P], F32)
                nc.sync.dma_start(out=identf[:], in_=ident32_in[:])
                # fold (1-w) of the local/global mix into the LN gain and bias
                nc.vector.tensor_scalar(out=g_t[:], in0=g_t[:], scalar1=w1m_col[:, 0:1],
                                        scalar2=None, op0=OP.mult)
                nc.vector.tensor_scalar(out=b_t[:], in0=b_t[:], scalar1=w1m_col[:, 0:1],
                                        scalar2=None, op0=OP.mult)
                fcb_t = ph4.tile([P, OUTC], F32)
                nc.sync.dma_start(out=fcb_t[:], in_=fcb_rep[:])

                # ---- softmax denominators: transpose to node-major via tiny
                # ones-matmuls, single cheap reciprocal; 1/den is applied
                # per-query inside the qt loop below.
                with tc.tile_pool(name="ps4a", bufs=1, space="PSUM") as ps4a:
                    on64 = ph4.tile([P, 1], BF16)
                    nc.vector.memset(on64[:], 1.0)
                    den_cols = ph4.tile([P, H * NT_LOC], F32)
                    for qt in range(NT_LOC):
                        for h in range(H):
                            tpc = ps4a.tile([P, 1], F32, tag="tpc", bufs=2)
                            nc.tensor.matmul(
                                out=tpc[:],
                                lhsT=OTu_sb[h][DH:DH + 1, qt * P:(qt + 1) * P],
                                rhs=on64[64:65, 0:1],
                                start=True, stop=True)
                            nc.vector.tensor_copy(
                                out=den_cols[:, qt * H + h:qt * H + h + 1], in_=tpc[:])
                    rden_cols = ph4.tile([P, H * NT_LOC], F32)
                    nc.vector.reciprocal(out=rden_cols[:], in_=den_cols[:])

                ps4 = tc.alloc_tile_pool(name="ps4", bufs=1, space="PSUM")

                def transpose_2chunks(src_ap, tag, on_act=False):
                    # f32 transpose with converting bf16 drains, split ACT/DVE
                    dst = ph4.tile([P, C], BF16, tag=tag, bufs=2, name=f"t2c{tag}")
                    for c in range(2):
                        tp = ps4.tile([P, P], F32, tag="tp", bufs=2)
                        nc.tensor.transpose(out=tp[:], in_=src_ap[:, c * P:(c + 1) * P],
                                            identity=identf[:])
                        if c == 0:
                            nc.scalar.copy(out=dst[:, c * P:(c + 1) * P], in_=tp[:])
                        else:
                            nc.vector.tensor_copy(out=dst[:, c * P:(c + 1) * P], in_=tp[:])
                    return dst

                for qt in range(NT_LOC):
                    # ---- global path: per-head out_proj, scaled by 1/den per
                    # query while summing heads (ACT/DVE split) ----
                    xo_t = ph4.tile([P, C], F32, tag="xot", bufs=2)
                    nc.sync.dma_start(out=xo_t[:], in_=x_own[qt * P:(qt + 1) * P, :])
                    cps = [ps4.tile([P, C], F32, tag="cps", bufs=4, name=f"cps{h}_{qt}")
                           for h in range(H)]
                    for h in range(H):
                        nc.tensor.matmul(
                            out=cps[h][:],
                            lhsT=OTu_sb[h][0:DH, qt * P:(qt + 1) * P],
                            rhs=Wop_t[:, h * C:(h + 1) * C],
                            start=True, stop=True)
                    vs = [ph4.tile([P, C], F32, tag="vs", bufs=8, name=f"vs{h}_{qt}")
                          for h in range(H)]
                    for h in range(H):
                        r_ap = rden_cols[:, qt * H + h:qt * H + h + 1]
                        if h % 2 == 0:
                            nc.scalar.activation(out=vs[h][:], in_=cps[h][:],
                                                 func=AF.Copy, scale=r_ap)
                        else:
                            nc.vector.tensor_scalar(out=vs[h][:], in0=cps[h][:],
                                                    scalar1=r_ap, scalar2=None,
                                                    op0=OP.mult)
                    xob = ph4.tile([P, C], F32, tag="xob", bufs=2)
                    nc.vector.tensor_tensor(out=xob[:], in0=xo_t[:], in1=opb_t[:],
                                            op=OP.add)
                    nc.vector.tensor_tensor(out=vs[0][:], in0=vs[0][:], in1=vs[1][:],
                                            op=OP.add)
                    nc.gpsimd.tensor_tensor(out=vs[2][:], in0=vs[2][:], in1=vs[3][:],
                                            op=OP.add)
                    nc.vector.tensor_tensor(out=vs[0][:], in0=vs[0][:], in1=vs[2][:],
                                            op=OP.add)
                    v_t = ph4.tile([P, C], F32, tag="vt", bufs=2)
                    nc.vector.tensor_tensor(out=v_t[:], in0=vs[0][:], in1=xob[:],
                                            op=OP.add)
                    msum = ph4.tile([P, 1], F32, tag="msum", bufs=2)
                    nc.vector.reduce_sum(out=msum[:], in_=v_t[:], axis=AX.X)
                    mean_neg = ph4.tile([P, 1], F32, tag="mean", bufs=2)
                    nc.vector.tensor_scalar(out=mean_neg[:], in0=msum[:],
                                            scalar1=-1.0 / C, scalar2=None, op0=OP.mult)
                    nc.scalar.activation(out=v_t[:], in_=v_t[:], func=AF.Identity,
                                         bias=mean_neg[:, 0:1])
                    sq = ph4.tile([P, C], F32, tag="sq", bufs=2)
                    ssum = ph4.tile([P, 1], F32, tag="ssum", bufs=2)
                    nc.scalar.activation(out=sq[:], in_=v_t[:], func=AF.Square,
                                         accum_out=ssum[:])
                    sstd = ph4.tile([P, 1], F32, tag="sstd", bufs=2)
                    nc.scalar.activation(out=sstd[:], in_=ssum[:], func=AF.Sqrt,
                                         bias=eps_col[:, 0:1], scale=1.0 / C)
                    rstd = ph4.tile([P, 1], F32, tag="rstd", bufs=2)
                    nc.vector.reciprocal(out=rstd[:], in_=sstd[:])
                    nc.scalar.activation(out=v_t[:], in_=v_t[:], func=AF.Copy,
                                         scale=rstd[:, 0:1])
                    nc.vector.tensor_tensor(out=v_t[:], in0=v_t[:], in1=g_t[:], op=OP.mult)
                    nc.vector.tensor_tensor(out=v_t[:], in0=v_t[:], in1=b_t[:], op=OP.add)

                    # ---- local path: local_embed = hi @ W_local ----
                    hiT = transpose_2chunks(hi_sb[qt][:], "hiT", on_act=True)
                    lps = ps4.tile([P, C], F32, tag="lps", bufs=2)
                    for c in range(2):
                        nc.tensor.matmul(out=lps[:], lhsT=hiT[:, c * P:(c + 1) * P],
                                         rhs=Wl_t[:, c * C:(c + 1) * C],
                                         start=(c == 0), stop=(c == 1))
                    # combined = w*local + (1-w)*global; the (1-w) factor is
                    # already inside v_t via the scaled LN gain/bias.
                    comb = ph4.tile([P, C], F32, tag="comb", bufs=2)
                    nc.scalar.activation(out=comb[:], in_=lps[:], func=AF.Copy,
                                         scale=w_col[:, 0:1])
                    nc.vector.tensor_tensor(out=comb[:], in0=comb[:], in1=v_t[:], op=OP.add)

                    # ---- fc ----
                    cT = transpose_2chunks(comb[:], "cT", on_act=True)
                    fps = ps4.tile([P, OUTC], F32, tag="lps", bufs=2, name=f"fps{qt}")
                    for c in range(2):
                        nc.tensor.matmul(out=fps[:], lhsT=cT[:, c * P:(c + 1) * P],
                                         rhs=fc_t[:, c * OUTC:(c + 1) * OUTC],
                                         start=(c == 0), stop=(c == 1))
                    o_t = ph4.tile([P, OUTC], F32, tag="ot", bufs=2)
                    nc.vector.tensor_tensor(out=o_t[:], in0=fps[:], in1=fcb_t[:], op=OP.add)
                    nc.sync.dma_start(out=out[qt * P:(qt + 1) * P, :], in_=o_t[:])
                ps4.release()
    nc.finalize()
    return nc


def _prep_edges(adj):
    """Bucket edges by destination node-tile; pad segments to a common length.

    Pure relayout/counting on the host: per-core local col indices, per-edge
    source-row indices (used to gather x into edge order), integer degrees.
    """
    row = np.asarray(adj[0], dtype=np.int64)
    col = np.asarray(adj[1], dtype=np.int64)
    d = np.bincount(col, minlength=N).astype(np.int32)
    tid = col // P
    order = np.argsort(tid, kind='stable')
    row_s, col_s = row[order], col[order]
    counts = np.bincount(tid, minlength=NT_GLOB)
    S = int(np.ceil(max(counts.max(), 1) / P) * P)
    TPT = S // P
    col_pad = np.full((NT_GLOB, S), -1, dtype=np.int32)
    row_pad = np.zeros((NT_GLOB, S), dtype=np.int32)
    start = 0
    for g in range(NT_GLOB):
        cnt = int(counts[g])
        col_pad[g, :cnt] = (col_s[start:start + cnt] - g * P).astype(np.int32)
        row_pad[g, :cnt] = row_s[start:start + cnt].astype(np.int32)
        start += cnt
    TE = NT_LOC * TPT
    per_core = []
    for k in range(NCORES):
        cols_k = col_pad[NT_LOC * k:NT_LOC * (k + 1)].reshape(TE, P)
        rows_k = row_pad[NT_LOC * k:NT_LOC * (k + 1)].reshape(TE, P)
        ca = np.ascontiguousarray(cols_k.T)                    # [P, TE]
        dre = np.ascontiguousarray(d[rows_k].T)                # [P, TE]
        down = np.ascontiguousarray(
            d[k * NPC:(k + 1) * NPC].reshape(NT_LOC, P).T)     # [P, NT_LOC]
        per_core.append((ca, rows_k.reshape(-1), dre, down))
    return per_core, TPT


def kernel(x, adj, weight_local, in_proj_w, in_proj_b, out_proj_w, out_proj_b,
           ln_g, ln_b, alpha, fc_w, fc_b):
    global LAST_RESULTS
    x = np.ascontiguousarray(np.asarray(x, dtype=np.float32))
    per_core_edges, TPT = _prep_edges(np.asarray(adj))

    bf = ml_dtypes.bfloat16
    x_bf = x.astype(bf)
    xT_bf = np.ascontiguousarray(x_bf.T)
    WopT = np.asarray(out_proj_w).T.astype(np.float32)  # [C_in, C_out]
    Wop4 = np.ascontiguousarray(
        WopT.reshape(H, DH, C).transpose(1, 0, 2).reshape(DH, H * C).astype(bf))
    common = dict(
        xT=xT_bf,
        WqT=np.ascontiguousarray(np.asarray(in_proj_w)[0:C].T.astype(bf)),
        WkT=np.ascontiguousarray(np.asarray(in_proj_w)[C:2 * C].T.astype(bf)),
        WvT=np.ascontiguousarray(np.asarray(in_proj_w)[2 * C:3 * C].T.astype(bf)),
        Wop4=Wop4,
        Wl=np.ascontiguousarray(np.asarray(weight_local).astype(bf)),
        fcT=np.ascontiguousarray(np.asarray(fc_w).T.astype(bf)),
        bq_pack=np.ascontiguousarray(np.asarray(in_proj_b)[0:C].astype(np.float32).reshape(2, P).T),
        bk_pack=np.ascontiguousarray(np.asarray(in_proj_b)[C:2 * C].astype(np.float32).reshape(2, P).T),
        bv_rep=np.tile(np.asarray(in_proj_b)[2 * C:3 * C].astype(np.float32), (P, 1)),
        opb_rep=np.tile(np.asarray(out_proj_b, dtype=np.float32), (P, 1)),
        g_rep=np.tile(np.asarray(ln_g, dtype=np.float32), (P, 1)),
        b_rep=np.tile(np.asarray(ln_b, dtype=np.float32), (P, 1)),
        fcb_rep=np.tile(np.asarray(fc_b, dtype=np.float32), (P, 1)),
        alpha11=np.asarray(alpha, dtype=np.float32).reshape(1, 1),
        iota_in=np.tile(np.arange(P, dtype=np.float32), (P, 1)).astype(bf),
        ident32_in=np.eye(P, dtype=np.float32),
        ident_in=np.eye(P, dtype=np.float32).astype(bf),
        ones_row_in=np.ones((1, P), dtype=np.float32),
    )
    in_maps = []
    for k in range(NCORES):
        ca, rows_flat, dre, down = per_core_edges[k]
        m = dict(common)
        m['xT_own'] = np.ascontiguousarray(xT_bf[:, k * NPC:(k + 1) * NPC])
        m['x_own'] = np.ascontiguousarray(x[k * NPC:(k + 1) * NPC, :])
        m['xe'] = np.ascontiguousarray(x_bf[rows_flat])
        m['col_adj'] = ca
        m['dr_edge'] = dre
        m['d_own'] = down
        in_maps.append(m)

    nc = _build(TPT)
    res = run_bass_kernel_spmd(nc, in_maps, core_ids=list(range(NCORES)))
    LAST_RESULTS = res
    return np.concatenate([res.results[k]['out'] for k in range(NCORES)], axis=0)


# revision 50
# speedup vs baseline: 1.0308x; 1.0066x over previous
"""Trainium2 Bass kernel for LocalGlobalEnvEncoder (GCN + MHA fusion).

Sharding: nodes are split across the 8 cores (1024 dest nodes / queries each).
 - GCN: edges bucketed by destination node-tile on host (layout only); source
   features are laid out in edge order on host (a pure gather / relayout), so
   the device streams them with plain sequential DMA instead of per-row
   indirect gathers. Per-edge 1/sqrt(d_row) weights are folded into the
   one-hot scatter matrix (built on DVE with a fused is_equal*mult chain) and
   the scatter-add runs on the PE in bf16.  Degrees are integer counts
   (host-side bincount relayout); all floating-point math (rsqrt, scaling,
   matmuls) happens on device.
 - MHA: query-sharded attention, K/V computed redundantly per core in bf16.
   Scores are kept transposed ([key, query]); exp runs on the ACT engine
   writing bf16; attn@V uses V as the stationary operand accumulating
   O^T [head_dim+1, queries] over key tiles, with softmax denominators coming
   from an appended ones-column in V.  Normalization (1/den) is applied
   per-query in node-major layout during the output projection.
All floating-point math happens on device; the host only re-lays-out inputs.
"""
import sys
sys.path.insert(0, '/opt/trn_rl_repo')
import numpy as np
import ml_dtypes
import concourse.bass as bass
import concourse.tile as tile
from concourse import bacc, mybir
from concourse.bass_utils import run_bass_kernel_spmd

F32 = mybir.dt.float32
BF16 = mybir.dt.bfloat16
I32 = mybir.dt.int32
AF = mybir.ActivationFunctionType
OP = mybir.AluOpType
AX = mybir.AxisListType

N, E, C, OUTC, H, DH = 8192, 262144, 256, 256, 4, 64
NCORES = 8
NPC = N // NCORES          # nodes per core = 1024
P = 128
NT_LOC = NPC // P          # node tiles per core = 8
NT_GLOB = N // P           # global node tiles = 64
EXP_BIAS = -12.0           # uniform shift inside softmax exp; cancels in the ratio

LAST_RESULTS = None        # stashed BassKernelResults for test harness introspection


def _build(TPT):
    """Build the single SPMD Bass program. TPT = edge tiles per node-tile segment."""
    nc = bacc.Bacc('TRN2', target_bir_lowering=False, debug=False, num_devices=NCORES)
    TE = NT_LOC * TPT  # total edge tiles per core

    # ---- I/O ----
    xT = nc.dram_tensor("xT", [C, N], BF16, kind="ExternalInput")
    xT_own = nc.dram_tensor("xT_own", [C, NPC], BF16, kind="ExternalInput")
    x_own = nc.dram_tensor("x_own", [NPC, C], F32, kind="ExternalInput")
    xe = nc.dram_tensor("xe", [TE * P, C], BF16, kind="ExternalInput")
    WqT = nc.dram_tensor("WqT", [C, C], BF16, kind="ExternalInput")
    WkT = nc.dram_tensor("WkT", [C, C], BF16, kind="ExternalInput")
    WvT = nc.dram_tensor("WvT", [C, C], BF16, kind="ExternalInput")
    Wop4 = nc.dram_tensor("Wop4", [DH, H * C], BF16, kind="ExternalInput")
    Wl = nc.dram_tensor("Wl", [C, C], BF16, kind="ExternalInput")
    fcT = nc.dram_tensor("fcT", [C, OUTC], BF16, kind="ExternalInput")
    bq_pack = nc.dram_tensor("bq_pack", [P, 2], F32, kind="ExternalInput")
    bk_pack = nc.dram_tensor("bk_pack", [P, 2], F32, kind="ExternalInput")
    bv_rep = nc.dram_tensor("bv_rep", [P, C], F32, kind="ExternalInput")
    opb_rep = nc.dram_tensor("opb_rep", [P, C], F32, kind="ExternalInput")
    g_rep = nc.dram_tensor("g_rep", [P, C], F32, kind="ExternalInput")
    b_rep = nc.dram_tensor("b_rep", [P, C], F32, kind="ExternalInput")
    fcb_rep = nc.dram_tensor("fcb_rep", [P, OUTC], F32, kind="ExternalInput")
    alpha11 = nc.dram_tensor("alpha11", [1, 1], F32, kind="ExternalInput")
    iota_in = nc.dram_tensor("iota_in", [P, P], BF16, kind="ExternalInput")
    ident_in = nc.dram_tensor("ident_in", [P, P], BF16, kind="ExternalInput")
    ident32_in = nc.dram_tensor("ident32_in", [P, P], F32, kind="ExternalInput")
    ones_row_in = nc.dram_tensor("ones_row_in", [1, P], F32, kind="ExternalInput")
    col_adj = nc.dram_tensor("col_adj", [P, TE], I32, kind="ExternalInput")
    dr_edge = nc.dram_tensor("dr_edge", [P, TE], I32, kind="ExternalInput")
    d_own = nc.dram_tensor("d_own", [P, NT_LOC], I32, kind="ExternalInput")

    out = nc.dram_tensor("out", [NPC, OUTC], F32, kind="ExternalOutput")

    with tile.TileContext(nc) as tc:
        with tc.tile_pool(name="const", bufs=1) as const:
            # phase-2-critical loads issue first on the sync queue
            Wq_t = const.tile([P, 2 * C], BF16)
            nc.sync.dma_start(out=Wq_t[:].rearrange("p (c n) -> p c n", c=2), in_=WqT[:].rearrange("(c p) n -> p c n", p=P))
            Wk_t = const.tile([P, 2 * C], BF16)
            nc.sync.dma_start(out=Wk_t[:].rearrange("p (c n) -> p c n", c=2), in_=WkT[:].rearrange("(c p) n -> p c n", p=P))
            Wv_t = const.tile([P, 2 * C], BF16)
            nc.sync.dma_start(out=Wv_t[:].rearrange("p (c n) -> p c n", c=2), in_=WvT[:].rearrange("(c p) n -> p c n", p=P))
            xo = [const.tile([P, NPC], BF16, tag=f"xo{c}", name=f"xo{c}") for c in range(2)]
            for c in range(2):
                nc.sync.dma_start(out=xo[c][:], in_=xT_own[c * P:(c + 1) * P, :])
            xts = [const.tile([P, N], BF16, name=f"xts{c}") for c in range(2)]
            for c in range(2):
                nc.sync.dma_start(out=xts[c][:, 0:N // 2],
                                  in_=xT[c * P:(c + 1) * P, 0:N // 2])
            for c in range(2):
                nc.sync.dma_start(out=xts[c][:, N // 2:N],
                                  in_=xT[c * P:(c + 1) * P, N // 2:N])
            bq_t = const.tile([P, 2], F32)
            nc.sync.dma_start(out=bq_t[:], in_=bq_pack[:])
            bk_t = const.tile([P, 2], F32)
            nc.sync.dma_start(out=bk_t[:], in_=bk_pack[:])
            bv_t = const.tile([P, C], F32)
            nc.sync.dma_start(out=bv_t[:], in_=bv_rep[:])

            # ---- persistent constants ----
            iota_t = const.tile([P, P], BF16)
            nc.sync.dma_start(out=iota_t[:], in_=iota_in[:])
            ident_t = const.tile([P, P], BF16)
            nc.sync.dma_start(out=ident_t[:], in_=ident_in[:])
            ones_row_t = const.tile([1, P], F32)
            nc.sync.dma_start(out=ones_row_t[:], in_=ones_row_in[:])
            col_t = const.tile([P, TE], I32)
            nc.sync.dma_start(out=col_t[:], in_=col_adj[:])
            colf_t = const.tile([P, TE], F32)
            nc.vector.tensor_copy(out=colf_t[:], in_=col_t[:])
            expb_col = const.tile([P, 1], F32)
            nc.vector.memset(expb_col[:], EXP_BIAS)
            eps_col = const.tile([P, 1], F32)
            nc.vector.memset(eps_col[:], 1e-5)
            w_col = const.tile([P, 1], F32)
            w1m_col = const.tile([P, 1], F32)

            # guarded rsqrt of integer degrees: d=0 -> 0, else 1/sqrt(d)
            rs_row = const.tile([P, TE], F32)
            s_own = const.tile([P, NT_LOC], F32)

            with tc.tile_pool(name="ph1", bufs=1) as ph1, \
                 tc.tile_pool(name="ps1", bufs=1, space="PSUM") as ps1:
                al_t = ph1.tile([1, 1], F32)
                nc.sync.dma_start(out=al_t[:], in_=alpha11[:])
                wsig = ph1.tile([1, 1], F32)
                nc.scalar.activation(out=wsig[:], in_=al_t[:], func=AF.Sigmoid)
                wrep_ps = ps1.tile([P, 1], F32, tag="wrep")
                nc.tensor.matmul(out=wrep_ps[:], lhsT=ones_row_t[:], rhs=wsig[:],
                                 start=True, stop=True)
                nc.vector.tensor_copy(out=w_col[:], in_=wrep_ps[:])
                nc.vector.tensor_scalar(out=w1m_col[:], in0=w_col[:], scalar1=-1.0,
                                        scalar2=1.0, op0=OP.mult, op1=OP.add)

                for (src_dram, dst, w_) in ((dr_edge, rs_row, TE), (d_own, s_own, NT_LOC)):
                    di = ph1.tile([P, w_], I32, tag=f"di{w_}")
                    nc.sync.dma_start(out=di[:], in_=src_dram[:])
                    df = ph1.tile([P, w_], F32, tag=f"df{w_}")
                    nc.vector.tensor_copy(out=df[:], in_=di[:])
                    m_t = ph1.tile([P, w_], F32, tag=f"m{w_}")
                    nc.vector.tensor_scalar(out=m_t[:], in0=df[:], scalar1=1.0,
                                            scalar2=None, op0=OP.min)
                    t1 = ph1.tile([P, w_], F32, tag=f"t1{w_}")
                    nc.vector.tensor_scalar(out=t1[:], in0=df[:], scalar1=1.0,
                                            scalar2=None, op0=OP.add)
                    nc.vector.tensor_tensor(out=t1[:], in0=t1[:], in1=m_t[:],
                                            op=OP.subtract)
                    nc.scalar.activation(out=t1[:], in_=t1[:], func=AF.Sqrt)
                    nc.vector.reciprocal(out=t1[:], in_=t1[:])
                    nc.vector.tensor_tensor(out=dst[:], in0=t1[:], in1=m_t[:],
                                            op=OP.mult)

            # ================= phase 2: QKV projections (bf16) =================
            big = const
            KTp = [big.tile([P, N], BF16, name=f"KT{p}") for p in range(2)]
            QTp = [big.tile([P, NPC], BF16, name=f"QT{p}") for p in range(2)]
            Vt = big.tile([P, NT_GLOB * H * (DH + 1)], BF16, name="Vt")
            V4 = Vt[:].rearrange("p (k h d) -> p k h d", h=H, d=DH + 1)
            OTu_sb = [big.tile([DH + 1, NPC], BF16, name=f"OTu{h}") for h in range(H)]
            hi_sb = [big.tile([P, C], F32, name=f"hi{i}") for i in range(NT_LOC)]

            nc.vector.memset(V4[:, :, :, DH:DH + 1], 1.0)  # ones column for denominators

            with tc.tile_pool(name="ph2", bufs=1) as ph2, \
                 tc.tile_pool(name="ps2", bufs=1, space="PSUM") as ps2:
                for p in range(2):
                    qps = ps2.tile([P, NPC], F32, tag="qkps", bufs=2)
                    for c in range(2):
                        for nb in range(NPC // 512):
                            mi = nc.tensor.matmul(
                                out=qps[:, nb * 512:(nb + 1) * 512],
                                lhsT=Wq_t[:, c * C + p * P: c * C + (p + 1) * P],
                                rhs=xo[c][:, nb * 512:(nb + 1) * 512],
                                start=(c == 0), stop=(c == 1))
                            if nb > 0:
                                mi.ins.ldweights = False  # same weight chunk
                    nc.scalar.activation(
                        out=QTp[p][:], in_=qps[:],
                        func=AF.Identity, bias=bq_t[:, p:p + 1])

                # K and V in slabs of 1024 nodes from the two resident xT halves
                SLAB = 1024
                for s in range(N // SLAB):
                    for p in range(2):
                        kps = ps2.tile([P, SLAB], F32, tag="qkps", bufs=2)
                        for c in range(2):
                            for nb in range(SLAB // 512):
                                nc.tensor.matmul(
                                    out=kps[:, nb * 512:(nb + 1) * 512],
                                    lhsT=Wk_t[:, c * C + p * P: c * C + (p + 1) * P],
                                    rhs=xts[c][:, s * SLAB + nb * 512:s * SLAB + (nb + 1) * 512],
                                    start=(c == 0), stop=(c == 1))
                        nc.scalar.activation(
                            out=KTp[p][:, s * SLAB:(s + 1) * SLAB],
                            in_=kps[:], func=AF.Identity, bias=bk_t[:, p:p + 1])
                    for ntl in range(SLAB // P):
                        g = s * (SLAB // P) + ntl
                        vps = ps2.tile([P, C], F32, tag="vps", bufs=2)
                        for c in range(2):
                            nc.tensor.matmul(
                                out=vps[:],
                                lhsT=xts[c][:, g * P:(g + 1) * P],
                                rhs=Wv_t[:, c * C:(c + 1) * C],
                                start=(c == 0), stop=(c == 1))
                        nc.vector.tensor_tensor(
                            out=V4[:, g, :, 0:DH],
                            in0=vps[:].rearrange("p (h d) -> p h d", d=DH),
                            in1=bv_t[:].rearrange("p (h d) -> p h d", d=DH),
                            op=OP.add)

            # ========== phase 3: attention + interleaved GCN scatter ==========
            with tc.tile_pool(name="ph3", bufs=1) as ph3, \
                 tc.tile_pool(name="ps3", bufs=1, space="PSUM") as ps3:

                # GCN scatter jobs, interleaved across attention steps so the
                # sequential xe DMA streams overlap attention compute.  xe is
                # loaded XB tiles per DMA to keep the SP queue issue rate low.
                XB = 8
                scat_jobs = [(t, i) for t in range(NT_LOC) for i in range(TPT)]
                n_jobs = len(scat_jobs)
                n_steps = H * NT_GLOB
                emitted = 0
                hips_cur = {}
                xeb_cur = [None]

                built = 0
                oh_ring = {}

                def emit_scatter_builds(upto):
                    # one-hot builds (DVE) and xe loads run AHEAD of their
                    # consuming matmuls so the PE never waits on fresh data.
                    nonlocal built
                    while built < min(upto, n_jobs):
                        j = built
                        if j % XB == 0:
                            nb = min(XB, TE - j)
                            xeb_cur[0] = ph3.tile([P, XB, C], BF16, tag="xet", bufs=3,
                                                  name=f"xeb{j}")
                            nc.sync.dma_start(
                                out=xeb_cur[0][:, 0:nb, :],
                                in_=xe[j * P:(j + nb) * P, :].rearrange(
                                    "(i p) c -> p i c", p=P))
                        # weighted one-hot: (iota == col) * rsqrt(d_row)
                        oh = ph3.tile([P, P], BF16, tag="oh2", bufs=10, name=f"oh{j}")
                        nc.vector.tensor_scalar(
                            out=oh[:], in0=iota_t[:], scalar1=colf_t[:, j:j + 1],
                            scalar2=rs_row[:, j:j + 1], op0=OP.is_equal, op1=OP.mult)
                        oh_ring[j] = (oh, xeb_cur[0])
                        built += 1

                def emit_scatter_jobs(upto):
                    nonlocal emitted
                    emit_scatter_builds(upto + 6)
                    while emitted < min(upto, n_jobs):
                        t, i = scat_jobs[emitted]
                        j = t * TPT + i
                        if i == 0:
                            hips_cur[t] = ps3.tile([P, C], F32, tag="hips", bufs=2, name=f"hips{t}")
                        oh, xeb = oh_ring.pop(j)
                        nc.tensor.matmul(out=hips_cur[t][:], lhsT=oh[:],
                                         rhs=xeb[:, j % XB, :],
                                         start=(i == 0), stop=(i == TPT - 1))
                        if i == TPT - 1:
                            nc.vector.tensor_scalar(out=hi_sb[t][:], in0=hips_cur[t][:],
                                                    scalar1=s_own[:, t:t + 1],
                                                    scalar2=None, op0=OP.mult)
                        emitted += 1

                for h in range(H):
                    p, hh = h // 2, h % 2
                    po = hh * DH
                    OT_ps = ps3.tile([DH + 1, NPC], F32, tag="OT", bufs=1, name=f"OTps{h}")

                    def emit_attnv(kt, et):
                        for qh in range(2):
                            mi = nc.tensor.matmul(
                                out=OT_ps[:, qh * 512:(qh + 1) * 512],
                                lhsT=V4[:, kt, h, :],
                                rhs=et[:, qh * 512:(qh + 1) * 512],
                                start=(kt == 0), stop=(kt == NT_GLOB - 1))
                            if qh == 1:
                                mi.ins.ldweights = False  # same V tile as qh=0

                    # software-pipelined: attn@V runs two steps behind exp, so
                    # the PE consumes exp outputs produced ~2.5 us earlier and
                    # never waits on semaphore propagation from the ACT engine.
                    DEPTH = 2
                    et_hist = []
                    for kt in range(NT_GLOB):
                        sps = ps3.tile([P, NPC], F32, tag="sps", bufs=2)
                        for qh in range(2):
                            nc.tensor.matmul(
                                out=sps[:, qh * 512:(qh + 1) * 512],
                                lhsT=KTp[p][po:po + DH, kt * P:(kt + 1) * P],
                                rhs=QTp[p][po:po + DH, qh * 512:(qh + 1) * 512],
                                start=True, stop=True)
                        et = ph3.tile([P, NPC], BF16, tag="expT", bufs=DEPTH + 2)
                        nc.scalar.activation(out=et[:], in_=sps[:], func=AF.Exp,
                                             bias=expb_col[:, 0:1], scale=1.0 / np.sqrt(DH))
                        et_hist.append(et)
                        step = h * NT_GLOB + kt + 1
                        emit_scatter_jobs(n_jobs * step // n_steps)
                        if kt >= DEPTH:
                            emit_attnv(kt - DEPTH, et_hist[kt - DEPTH])
                    for kt in range(NT_GLOB - DEPTH, NT_GLOB):
                        emit_attnv(kt, et_hist[kt])

                    # drain PSUM (releases the accumulator); normalization by
                    # the denominator happens at the start of phase 4.
                    nc.vector.tensor_copy(out=OTu_sb[h][:], in_=OT_ps[:])

            # ================= phase 4: out_proj, LN, combine, fc =================
            with tc.tile_pool(name="ph4", bufs=1) as ph4:
                Wop_t = ph4.tile([DH, H * C], BF16)
                nc.sync.dma_start(out=Wop_t[:], in_=Wop4[:])
                Wl_t = ph4.tile([P, 2 * C], BF16)
                nc.sync.dma_start(out=Wl_t[:].rearrange("p (c n) -> p c n", c=2), in_=Wl[:].rearrange("(c p) n -> p c n", p=P))
                fc_t = ph4.tile([P, 2 * OUTC], BF16)
                nc.sync.dma_start(out=fc_t[:].rearrange("p (c n) -> p c n", c=2), in_=fcT[:].rearrange("(c p) n -> p c n", p=P))
                opb_t = ph4.tile([P, C], F32)
                nc.sync.dma_start(out=opb_t[:], in_=opb_rep[:])
                g_t = ph4.tile([P, C], F32)
                nc.sync.dma_start(out=g_t[:], in_=g_rep[:])
                b_t = ph4.tile([P, C], F32)
                nc.sync.dma_start(out=b_t[:], in_=b_rep[:])
                identf = ph4.tile([P, P], F32)
                nc.sync.dma_start(out=identf[:], in_=ident32_in[:])
                # fold (1-w) of the local/global mix into the LN gain and bias
                nc.vector.tensor_scalar(out=g_t[:], in0=g_t[:], scalar1=w1m_col[:, 0:1],
                                        scalar2=None, op0=OP.mult)
                nc.vector.tensor_scalar(out=b_t[:], in0=b_t[:], scalar1=w1m_col[:, 0:1],
                                        scalar2=None, op0=OP.mult)
                fcb_t = ph4.tile([P, OUTC], F32)
                nc.sync.dma_start(out=fcb_t[:], in_=fcb_rep[:])

                # ---- softmax denominators: transpose to node-major via tiny
                # ones-matmuls, single cheap reciprocal; 1/den is applied
                # per-query inside the qt loop below.
                with tc.tile_pool(name="ps4a", bufs=1, space="PSUM") as ps4a:
                    on64 = ph4.tile([P, 1], BF16)
                    nc.vector.memset(on64[:], 1.0)
                    # all 32 per-(qt,head) denominator columns land in one PSUM
                    # tile (separate single-column matmul groups) -> one drain
                    dps = ps4a.tile([P, H * NT_LOC], F32, tag="dps", bufs=1)
                    for qt in range(NT_LOC):
                        for h in range(H):
                            nc.tensor.matmul(
                                out=dps[:, qt * H + h:qt * H + h + 1],
                                lhsT=OTu_sb[h][DH:DH + 1, qt * P:(qt + 1) * P],
                                rhs=on64[64:65, 0:1],
                                start=True, stop=True)
                    den_cols = ph4.tile([P, H * NT_LOC], F32)
                    nc.vector.tensor_copy(out=den_cols[:], in_=dps[:])
                    rden_cols = ph4.tile([P, H * NT_LOC], F32)
                    nc.vector.reciprocal(out=rden_cols[:], in_=den_cols[:])

                ps4 = tc.alloc_tile_pool(name="ps4", bufs=1, space="PSUM")

                def transpose_2chunks(src_ap, tag, on_act=False):
                    # f32 transpose with converting bf16 drains, split ACT/DVE
                    dst = ph4.tile([P, C], BF16, tag=tag, bufs=2, name=f"t2c{tag}")
                    for c in range(2):
                        tp = ps4.tile([P, P], F32, tag="tp", bufs=2)
                        nc.tensor.transpose(out=tp[:], in_=src_ap[:, c * P:(c + 1) * P],
                                            identity=identf[:])
                        if c == 0:
                            nc.scalar.copy(out=dst[:, c * P:(c + 1) * P], in_=tp[:])
                        else:
                            nc.vector.tensor_copy(out=dst[:, c * P:(c + 1) * P], in_=tp[:])
                    return dst

                for qt in range(NT_LOC):
                    # ---- global path: per-head out_proj, scaled by 1/den per
                    # query while summing heads (ACT/DVE split) ----
                    xo_t = ph4.tile([P, C], F32, tag="xot", bufs=2)
                    nc.sync.dma_start(out=xo_t[:], in_=x_own[qt * P:(qt + 1) * P, :])
                    cps = [ps4.tile([P, C], F32, tag="cps", bufs=4, name=f"cps{h}_{qt}")
                           for h in range(H)]
                    for h in range(H):
                        nc.tensor.matmul(
                            out=cps[h][:],
                            lhsT=OTu_sb[h][0:DH, qt * P:(qt + 1) * P],
                            rhs=Wop_t[:, h * C:(h + 1) * C],
                            start=True, stop=True)
                    vs = [ph4.tile([P, C], F32, tag="vs", bufs=8, name=f"vs{h}_{qt}")
                          for h in range(H)]
                    for h in range(H):
                        r_ap = rden_cols[:, qt * H + h:qt * H + h + 1]
                        if h % 2 == 0:
                            nc.scalar.activation(out=vs[h][:], in_=cps[h][:],
                                                 func=AF.Copy, scale=r_ap)
                        else:
                            nc.vector.tensor_scalar(out=vs[h][:], in0=cps[h][:],
                                                    scalar1=r_ap, scalar2=None,
                                                    op0=OP.mult)
                    xob = ph4.tile([P, C], F32, tag="xob", bufs=2)
                    nc.vector.tensor_tensor(out=xob[:], in0=xo_t[:], in1=opb_t[:],
                                            op=OP.add)
                    nc.vector.tensor_tensor(out=vs[0][:], in0=vs[0][:], in1=vs[1][:],
                                            op=OP.add)
                    nc.gpsimd.tensor_tensor(out=vs[2][:], in0=vs[2][:], in1=vs[3][:],
                                            op=OP.add)
                    nc.vector.tensor_tensor(out=vs[0][:], in0=vs[0][:], in1=vs[2][:],
                                            op=OP.add)
                    v_t = ph4.tile([P, C], F32, tag="vt", bufs=2)
                    nc.vector.tensor_tensor(out=v_t[:], in0=vs[0][:], in1=xob[:],
                                            op=OP.add)
                    msum = ph4.tile([P, 1], F32, tag="msum", bufs=2)
                    nc.vector.reduce_sum(out=msum[:], in_=v_t[:], axis=AX.X)
                    mean_neg = ph4.tile([P, 1], F32, tag="mean", bufs=2)
                    nc.vector.tensor_scalar(out=mean_neg[:], in0=msum[:],
                                            scalar1=-1.0 / C, scalar2=None, op0=OP.mult)
                    nc.scalar.activation(out=v_t[:], in_=v_t[:], func=AF.Identity,
                                         bias=mean_neg[:, 0:1])
                    sq = ph4.tile([P, C], F32, tag="sq", bufs=2)
                    ssum = ph4.tile([P, 1], F32, tag="ssum", bufs=2)
                    nc.scalar.activation(out=sq[:], in_=v_t[:], func=AF.Square,
                                         accum_out=ssum[:])
                    sstd = ph4.tile([P, 1], F32, tag="sstd", bufs=2)
                    nc.scalar.activation(out=sstd[:], in_=ssum[:], func=AF.Sqrt,
                                         bias=eps_col[:, 0:1], scale=1.0 / C)
                    rstd = ph4.tile([P, 1], F32, tag="rstd", bufs=2)
                    nc.vector.reciprocal(out=rstd[:], in_=sstd[:])
                    nc.scalar.activation(out=v_t[:], in_=v_t[:], func=AF.Copy,
                                         scale=rstd[:, 0:1])
                    nc.vector.tensor_tensor(out=v_t[:], in0=v_t[:], in1=g_t[:], op=OP.mult)
                    nc.vector.tensor_tensor(out=v_t[:], in0=v_t[:], in1=b_t[:], op=OP.add)

                    # ---- local path: local_embed = hi @ W_local ----
                    hiT = transpose_2chunks(hi_sb[qt][:], "hiT", on_act=True)
                    lps = ps4.tile([P, C], F32, tag="lps", bufs=2)
                    for c in range(2):
                        nc.tensor.matmul(out=lps[:], lhsT=hiT[:, c * P:(c + 1) * P],
                                         rhs=Wl_t[:, c * C:(c + 1) * C],
                                         start=(c == 0), stop=(c == 1))
                    # combined = w*local + (1-w)*global; the (1-w) factor is
                    # already inside v_t via the scaled LN gain/bias.
                    comb = ph4.tile([P, C], F32, tag="comb", bufs=2)
                    nc.scalar.activation(out=comb[:], in_=lps[:], func=AF.Copy,
                                         scale=w_col[:, 0:1])
                    nc.vector.tensor_tensor(out=comb[:], in0=comb[:], in1=v_t[:], op=OP.add)

                    # ---- fc ----
                    cT = transpose_2chunks(comb[:], "cT", on_act=True)
                    fps = ps4.tile([P, OUTC], F32, tag="lps", bufs=2, name=f"fps{qt}")
                    for c in range(2):
                        nc.tensor.matmul(out=fps[:], lhsT=cT[:, c * P:(c + 1) * P],
                                         rhs=fc_t[:, c * OUTC:(c + 1) * OUTC],
                                         start=(c == 0), stop=(c == 1))
                    o_t = ph4.tile([P, OUTC], F32, tag="ot", bufs=2)
                    nc.vector.tensor_tensor(out=o_t[:], in0=fps[:], in1=fcb_t[:], op=OP.add)
                    nc.sync.dma_start(out=out[qt * P:(qt + 1) * P, :], in_=o_t[:])
                ps4.release()
    nc.finalize()
    return nc


def _prep_edges(adj):
    """Bucket edges by destination node-tile; pad segments to a common length.

    Pure relayout/counting on the host: per-core local col indices, per-edge
    source-row indices (used to gather x into edge order), integer degrees.
    """
    row = np.asarray(adj[0], dtype=np.int64)
    col = np.asarray(adj[1], dtype=np.int64)
    d = np.bincount(col, minlength=N).astype(np.int32)
    tid = col // P
    order = np.argsort(tid, kind='stable')
    row_s, col_s = row[order], col[order]
    counts = np.bincount(tid, minlength=NT_GLOB)
    S = int(np.ceil(max(counts.max(), 1) / P) * P)
    TPT = S // P
    col_pad = np.full((NT_GLOB, S), -1, dtype=np.int32)
    row_pad = np.zeros((NT_GLOB, S), dtype=np.int32)
    start = 0
    for g in range(NT_GLOB):
        cnt = int(counts[g])
        col_pad[g, :cnt] = (col_s[start:start + cnt] - g * P).astype(np.int32)
        row_pad[g, :cnt] = row_s[start:start + cnt].astype(np.int32)
        start += cnt
    TE = NT_LOC * TPT
    per_core = []
    for k in range(NCORES):
        cols_k = col_pad[NT_LOC * k:NT_LOC * (k + 1)].reshape(TE, P)
        rows_k = row_pad[NT_LOC * k:NT_LOC * (k + 1)].reshape(TE, P)
        ca = np.ascontiguousarray(cols_k.T)                    # [P, TE]
        dre = np.ascontiguousarray(d[rows_k].T)                # [P, TE]
        down = np.ascontiguousarray(
            d[k * NPC:(k + 1) * NPC].reshape(NT_LOC, P).T)     # [P, NT_LOC]
        per_core.append((ca, rows_k.reshape(-1), dre, down))
    return per_core, TPT


def kernel(x, adj, weight_local, in_proj_w, in_proj_b, out_proj_w, out_proj_b,
           ln_g, ln_b, alpha, fc_w, fc_b):
    global LAST_RESULTS
    x = np.ascontiguousarray(np.asarray(x, dtype=np.float32))
    per_core_edges, TPT = _prep_edges(np.asarray(adj))

    bf = ml_dtypes.bfloat16
    x_bf = x.astype(bf)
    xT_bf = np.ascontiguousarray(x_bf.T)
    WopT = np.asarray(out_proj_w).T.astype(np.float32)  # [C_in, C_out]
    Wop4 = np.ascontiguousarray(
        WopT.reshape(H, DH, C).transpose(1, 0, 2).reshape(DH, H * C).astype(bf))
    common = dict(
        xT=xT_bf,
        WqT=np.ascontiguousarray(np.asarray(in_proj_w)[0:C].T.astype(bf)),
        WkT=np.ascontiguousarray(np.asarray(in_proj_w)[C:2 * C].T.astype(bf)),
        WvT=np.ascontiguousarray(np.asarray(in_proj_w)[2 * C:3 * C].T.astype(bf)),
        Wop4=Wop4,
        Wl=np.ascontiguousarray(np.asarray(weight_local).astype(bf)),
        fcT=np.ascontiguousarray(np.asarray(fc_w).T.astype(bf)),
        bq_pack=np.ascontiguousarray(np.asarray(in_proj_b)[0:C].astype(np.float32).reshape(2, P).T),
        bk_pack=np.ascontiguousarray(np.asarray(in_proj_b)[C:2 * C].astype(np.float32).reshape(2, P).T),
        bv_rep=np.tile(np.asarray(in_proj_b)[2 * C:3 * C].astype(np.float32), (P, 1)),
        opb_rep=np.tile(np.asarray(out_proj_b, dtype=np.float32), (P, 1)),
        g_rep=np.tile(np.asarray(ln_g, dtype=np.float32), (P, 1)),
        b_rep=np.tile(np.asarray(ln_b, dtype=np.float32), (P, 1)),
        fcb_rep=np.tile(np.asarray(fc_b, dtype=np.float32), (P, 1)),
        alpha11=np.asarray(alpha, dtype=np.float32).reshape(1, 1),
        iota_in=np.tile(np.arange(P, dtype=np.float32), (P, 1)).astype(bf),
        ident32_in=np.eye(P, dtype=np.float32),
        ident_in=np.eye(P, dtype=np.float32).astype(bf),
        ones_row_in=np.ones((1, P), dtype=np.float32),
    )
    in_maps = []
    for k in range(NCORES):
        ca, rows_flat, dre, down = per_core_edges[k]
        m = dict(common)
        m['xT_own'] = np.ascontiguousarray(xT_bf[:, k * NPC:(k + 1) * NPC])
        m['x_own'] = np.ascontiguousarray(x[k * NPC:(k + 1) * NPC, :])
        m['xe'] = np.ascontiguousarray(x_bf[rows_flat])
        m['col_adj'] = ca
        m['dr_edge'] = dre
        m['d_own'] = down
        in_maps.append(m)

    nc = _build(TPT)
    res = run_bass_kernel_spmd(nc, in_maps, core_ids=list(range(NCORES)))
    LAST_RESULTS = res
    return np.concatenate([res.results[k]['out'] for k in range(NCORES)], axis=0)


# revision 51
# speedup vs baseline: 1.0437x; 1.0125x over previous
"""Trainium2 Bass kernel for LocalGlobalEnvEncoder (GCN + MHA fusion).

Sharding: nodes are split across the 8 cores (1024 dest nodes / queries each).
 - GCN: edges bucketed by destination node-tile on host (layout only); source
   features are laid out in edge order on host (a pure gather / relayout), so
   the device streams them with plain sequential DMA instead of per-row
   indirect gathers. Per-edge 1/sqrt(d_row) weights are folded into the
   one-hot scatter matrix (built on DVE with a fused is_equal*mult chain) and
   the scatter-add runs on the PE in bf16.  Degrees are integer counts
   (host-side bincount relayout); all floating-point math (rsqrt, scaling,
   matmuls) happens on device.
 - MHA: query-sharded attention, K/V computed redundantly per core in bf16.
   Scores are kept transposed ([key, query]); exp runs on the ACT engine
   writing bf16; attn@V uses V as the stationary operand accumulating
   O^T [head_dim+1, queries] over key tiles, with softmax denominators coming
   from an appended ones-column in V.  Normalization (1/den) is applied
   per-query in node-major layout during the output projection.
All floating-point math happens on device; the host only re-lays-out inputs.
"""
import sys
sys.path.insert(0, '/opt/trn_rl_repo')
import numpy as np
import ml_dtypes
import concourse.bass as bass
import concourse.tile as tile
from concourse import bacc, mybir
from concourse.bass_utils import run_bass_kernel_spmd

F32 = mybir.dt.float32
BF16 = mybir.dt.bfloat16
I32 = mybir.dt.int32
AF = mybir.ActivationFunctionType
OP = mybir.AluOpType
AX = mybir.AxisListType

N, E, C, OUTC, H, DH = 8192, 262144, 256, 256, 4, 64
NCORES = 8
NPC = N // NCORES          # nodes per core = 1024
P = 128
NT_LOC = NPC // P          # node tiles per core = 8
NT_GLOB = N // P           # global node tiles = 64
EXP_BIAS = -12.0           # uniform shift inside softmax exp; cancels in the ratio

LAST_RESULTS = None        # stashed BassKernelResults for test harness introspection


def _build(TPT):
    """Build the single SPMD Bass program. TPT = edge tiles per node-tile segment."""
    nc = bacc.Bacc('TRN2', target_bir_lowering=False, debug=False, num_devices=NCORES)
    TE = NT_LOC * TPT  # total edge tiles per core

    # ---- I/O ----
    xT = nc.dram_tensor("xT", [C, N], BF16, kind="ExternalInput")
    xT_own = nc.dram_tensor("xT_own", [C, NPC], BF16, kind="ExternalInput")
    x_own = nc.dram_tensor("x_own", [NPC, C], F32, kind="ExternalInput")
    xe = nc.dram_tensor("xe", [TE * P, C], BF16, kind="ExternalInput")
    WqT = nc.dram_tensor("WqT", [C, C], BF16, kind="ExternalInput")
    WkT = nc.dram_tensor("WkT", [C, C], BF16, kind="ExternalInput")
    WvT = nc.dram_tensor("WvT", [C, C], BF16, kind="ExternalInput")
    Wop4 = nc.dram_tensor("Wop4", [DH, H * C], BF16, kind="ExternalInput")
    Wl = nc.dram_tensor("Wl", [C, C], BF16, kind="ExternalInput")
    fcT = nc.dram_tensor("fcT", [C, OUTC], BF16, kind="ExternalInput")
    bq_pack = nc.dram_tensor("bq_pack", [P, 2], F32, kind="ExternalInput")
    bk_pack = nc.dram_tensor("bk_pack", [P, 2], F32, kind="ExternalInput")
    bv_rep = nc.dram_tensor("bv_rep", [P, C], F32, kind="ExternalInput")
    opb_rep = nc.dram_tensor("opb_rep", [P, C], F32, kind="ExternalInput")
    g_rep = nc.dram_tensor("g_rep", [P, C], F32, kind="ExternalInput")
    b_rep = nc.dram_tensor("b_rep", [P, C], F32, kind="ExternalInput")
    fcb_rep = nc.dram_tensor("fcb_rep", [P, OUTC], F32, kind="ExternalInput")
    alpha11 = nc.dram_tensor("alpha11", [1, 1], F32, kind="ExternalInput")
    iota_in = nc.dram_tensor("iota_in", [P, P], BF16, kind="ExternalInput")
    ident_in = nc.dram_tensor("ident_in", [P, P], BF16, kind="ExternalInput")
    ident32_in = nc.dram_tensor("ident32_in", [P, P], F32, kind="ExternalInput")
    ones_row_in = nc.dram_tensor("ones_row_in", [1, P], F32, kind="ExternalInput")
    col_adj = nc.dram_tensor("col_adj", [P, TE], I32, kind="ExternalInput")
    dr_edge = nc.dram_tensor("dr_edge", [P, TE], I32, kind="ExternalInput")
    d_own = nc.dram_tensor("d_own", [P, NT_LOC], I32, kind="ExternalInput")

    out = nc.dram_tensor("out", [NPC, OUTC], F32, kind="ExternalOutput")

    with tile.TileContext(nc) as tc:
        with tc.tile_pool(name="const", bufs=1) as const:
            # phase-2-critical loads issue first on the sync queue
            Wq_t = const.tile([P, 2 * C], BF16)
            nc.sync.dma_start(out=Wq_t[:].rearrange("p (c n) -> p c n", c=2), in_=WqT[:].rearrange("(c p) n -> p c n", p=P))
            Wk_t = const.tile([P, 2 * C], BF16)
            nc.sync.dma_start(out=Wk_t[:].rearrange("p (c n) -> p c n", c=2), in_=WkT[:].rearrange("(c p) n -> p c n", p=P))
            Wv_t = const.tile([P, 2 * C], BF16)
            nc.sync.dma_start(out=Wv_t[:].rearrange("p (c n) -> p c n", c=2), in_=WvT[:].rearrange("(c p) n -> p c n", p=P))
            xo = [const.tile([P, NPC], BF16, tag=f"xo{c}", name=f"xo{c}") for c in range(2)]
            for c in range(2):
                nc.sync.dma_start(out=xo[c][:], in_=xT_own[c * P:(c + 1) * P, :])
            xts = [const.tile([P, N], BF16, name=f"xts{c}") for c in range(2)]
            for c in range(2):
                nc.sync.dma_start(out=xts[c][:, 0:N // 2],
                                  in_=xT[c * P:(c + 1) * P, 0:N // 2])
            for c in range(2):
                nc.sync.dma_start(out=xts[c][:, N // 2:N],
                                  in_=xT[c * P:(c + 1) * P, N // 2:N])
            bq_t = const.tile([P, 2], F32)
            nc.sync.dma_start(out=bq_t[:], in_=bq_pack[:])
            bk_t = const.tile([P, 2], F32)
            nc.sync.dma_start(out=bk_t[:], in_=bk_pack[:])
            bv_t = const.tile([P, C], F32)
            nc.sync.dma_start(out=bv_t[:], in_=bv_rep[:])

            # ---- persistent constants ----
            iota_t = const.tile([P, P], BF16)
            nc.sync.dma_start(out=iota_t[:], in_=iota_in[:])
            ident_t = const.tile([P, P], BF16)
            nc.sync.dma_start(out=ident_t[:], in_=ident_in[:])
            ones_row_t = const.tile([1, P], F32)
            nc.sync.dma_start(out=ones_row_t[:], in_=ones_row_in[:])
            col_t = const.tile([P, TE], I32)
            nc.sync.dma_start(out=col_t[:], in_=col_adj[:])
            colf_t = const.tile([P, TE], F32)
            nc.vector.tensor_copy(out=colf_t[:], in_=col_t[:])
            expb_col = const.tile([P, 1], F32)
            nc.vector.memset(expb_col[:], EXP_BIAS)
            eps_col = const.tile([P, 1], F32)
            nc.vector.memset(eps_col[:], 1e-5)
            w_col = const.tile([P, 1], F32)
            w1m_col = const.tile([P, 1], F32)

            # guarded rsqrt of integer degrees: d=0 -> 0, else 1/sqrt(d)
            rs_row = const.tile([P, TE], F32)
            s_own = const.tile([P, NT_LOC], F32)

            with tc.tile_pool(name="ph1", bufs=1) as ph1, \
                 tc.tile_pool(name="ps1", bufs=1, space="PSUM") as ps1:
                al_t = ph1.tile([1, 1], F32)
                nc.sync.dma_start(out=al_t[:], in_=alpha11[:])
                wsig = ph1.tile([1, 1], F32)
                nc.scalar.activation(out=wsig[:], in_=al_t[:], func=AF.Sigmoid)
                wrep_ps = ps1.tile([P, 1], F32, tag="wrep")
                nc.tensor.matmul(out=wrep_ps[:], lhsT=ones_row_t[:], rhs=wsig[:],
                                 start=True, stop=True)
                nc.vector.tensor_copy(out=w_col[:], in_=wrep_ps[:])
                nc.vector.tensor_scalar(out=w1m_col[:], in0=w_col[:], scalar1=-1.0,
                                        scalar2=1.0, op0=OP.mult, op1=OP.add)

                for (src_dram, dst, w_) in ((dr_edge, rs_row, TE), (d_own, s_own, NT_LOC)):
                    di = ph1.tile([P, w_], I32, tag=f"di{w_}")
                    nc.sync.dma_start(out=di[:], in_=src_dram[:])
                    df = ph1.tile([P, w_], F32, tag=f"df{w_}")
                    nc.vector.tensor_copy(out=df[:], in_=di[:])
                    m_t = ph1.tile([P, w_], F32, tag=f"m{w_}")
                    nc.vector.tensor_scalar(out=m_t[:], in0=df[:], scalar1=1.0,
                                            scalar2=None, op0=OP.min)
                    t1 = ph1.tile([P, w_], F32, tag=f"t1{w_}")
                    nc.vector.tensor_scalar(out=t1[:], in0=df[:], scalar1=1.0,
                                            scalar2=None, op0=OP.add)
                    nc.vector.tensor_tensor(out=t1[:], in0=t1[:], in1=m_t[:],
                                            op=OP.subtract)
                    nc.scalar.activation(out=t1[:], in_=t1[:], func=AF.Sqrt)
                    nc.vector.reciprocal(out=t1[:], in_=t1[:])
                    nc.vector.tensor_tensor(out=dst[:], in0=t1[:], in1=m_t[:],
                                            op=OP.mult)

            # ================= phase 2: QKV projections (bf16) =================
            big = const
            KTp = [big.tile([P, N], BF16, name=f"KT{p}") for p in range(2)]
            QTp = [big.tile([P, NPC], BF16, name=f"QT{p}") for p in range(2)]
            Vt = big.tile([P, NT_GLOB * H * (DH + 1)], BF16, name="Vt")
            V4 = Vt[:].rearrange("p (k h d) -> p k h d", h=H, d=DH + 1)
            OTu_sb = [big.tile([DH + 1, NPC], BF16, name=f"OTu{h}") for h in range(H)]
            hi_sb = [big.tile([P, C], F32, name=f"hi{i}") for i in range(NT_LOC)]

            nc.vector.memset(V4[:, :, :, DH:DH + 1], 1.0)  # ones column for denominators

            with tc.tile_pool(name="ph2", bufs=1) as ph2, \
                 tc.tile_pool(name="ps2", bufs=1, space="PSUM") as ps2:
                for p in range(2):
                    qps = ps2.tile([P, NPC], F32, tag="qkps", bufs=2)
                    for c in range(2):
                        for nb in range(NPC // 512):
                            mi = nc.tensor.matmul(
                                out=qps[:, nb * 512:(nb + 1) * 512],
                                lhsT=Wq_t[:, c * C + p * P: c * C + (p + 1) * P],
                                rhs=xo[c][:, nb * 512:(nb + 1) * 512],
                                start=(c == 0), stop=(c == 1))
                            if nb > 0:
                                mi.ins.ldweights = False  # same weight chunk
                    nc.scalar.activation(
                        out=QTp[p][:], in_=qps[:],
                        func=AF.Identity, bias=bq_t[:, p:p + 1])

                # K and V in slabs of 1024 nodes from the two resident xT halves
                SLAB = 1024
                for s in range(N // SLAB):
                    for p in range(2):
                        kps = ps2.tile([P, SLAB], F32, tag="qkps", bufs=2)
                        for c in range(2):
                            for nb in range(SLAB // 512):
                                nc.tensor.matmul(
                                    out=kps[:, nb * 512:(nb + 1) * 512],
                                    lhsT=Wk_t[:, c * C + p * P: c * C + (p + 1) * P],
                                    rhs=xts[c][:, s * SLAB + nb * 512:s * SLAB + (nb + 1) * 512],
                                    start=(c == 0), stop=(c == 1))
                        nc.scalar.activation(
                            out=KTp[p][:, s * SLAB:(s + 1) * SLAB],
                            in_=kps[:], func=AF.Identity, bias=bk_t[:, p:p + 1])
                    for ntl in range(SLAB // P):
                        g = s * (SLAB // P) + ntl
                        vps = ps2.tile([P, C], F32, tag="vps", bufs=2)
                        for c in range(2):
                            nc.tensor.matmul(
                                out=vps[:],
                                lhsT=xts[c][:, g * P:(g + 1) * P],
                                rhs=Wv_t[:, c * C:(c + 1) * C],
                                start=(c == 0), stop=(c == 1))
                        nc.vector.tensor_tensor(
                            out=V4[:, g, :, 0:DH],
                            in0=vps[:].rearrange("p (h d) -> p h d", d=DH),
                            in1=bv_t[:].rearrange("p (h d) -> p h d", d=DH),
                            op=OP.add)

            # ========== phase 3: attention + interleaved GCN scatter ==========
            with tc.tile_pool(name="ph3", bufs=1) as ph3, \
                 tc.tile_pool(name="ps3", bufs=1, space="PSUM") as ps3:
                ps3h = tc.alloc_tile_pool(name="ps3h", bufs=1, space="PSUM")
                ps3c = [None]

                # GCN scatter jobs, interleaved across attention steps so the
                # sequential xe DMA streams overlap attention compute.  xe is
                # loaded XB tiles per DMA to keep the SP queue issue rate low.
                XB = 8
                scat_jobs = [(t, i) for t in range(NT_LOC) for i in range(TPT)]
                n_jobs = len(scat_jobs)
                n_steps = 2 * NT_GLOB  # scatter finishes within heads 0-1
                emitted = 0
                hips_cur = {}
                xeb_cur = [None]

                built = 0
                oh_ring = {}

                def emit_scatter_builds(upto):
                    # one-hot builds (DVE) and xe loads run AHEAD of their
                    # consuming matmuls so the PE never waits on fresh data.
                    nonlocal built
                    while built < min(upto, n_jobs):
                        j = built
                        if j % XB == 0:
                            nb = min(XB, TE - j)
                            xeb_cur[0] = ph3.tile([P, XB, C], BF16, tag="xet", bufs=3,
                                                  name=f"xeb{j}")
                            nc.sync.dma_start(
                                out=xeb_cur[0][:, 0:nb, :],
                                in_=xe[j * P:(j + nb) * P, :].rearrange(
                                    "(i p) c -> p i c", p=P))
                        # weighted one-hot: (iota == col) * rsqrt(d_row)
                        oh = ph3.tile([P, P], BF16, tag="oh2", bufs=10, name=f"oh{j}")
                        nc.vector.tensor_scalar(
                            out=oh[:], in0=iota_t[:], scalar1=colf_t[:, j:j + 1],
                            scalar2=rs_row[:, j:j + 1], op0=OP.is_equal, op1=OP.mult)
                        oh_ring[j] = (oh, xeb_cur[0])
                        built += 1

                def emit_scatter_jobs(upto):
                    nonlocal emitted
                    emit_scatter_builds(upto + 6)
                    while emitted < min(upto, n_jobs):
                        t, i = scat_jobs[emitted]
                        j = t * TPT + i
                        if i == 0:
                            hips_cur[t] = ps3h.tile([P, C], F32, tag="hips", bufs=2, name=f"hips{t}")
                        oh, xeb = oh_ring.pop(j)
                        nc.tensor.matmul(out=hips_cur[t][:], lhsT=oh[:],
                                         rhs=xeb[:, j % XB, :],
                                         start=(i == 0), stop=(i == TPT - 1))
                        if i == TPT - 1:
                            nc.vector.tensor_scalar(out=hi_sb[t][:], in0=hips_cur[t][:],
                                                    scalar1=s_own[:, t:t + 1],
                                                    scalar2=None, op0=OP.mult)
                        emitted += 1

                for h in range(H):
                    p, hh = h // 2, h % 2
                    po = hh * DH
                    if h == 2:
                        # scatter is done; its two PSUM banks become a third
                        # sps ring slot so score matmuls wait on exp(kt-2)
                        # instead of exp(kt-1) (stale sem -> no PE latency).
                        ps3h.release()
                        ps3c[0] = tc.alloc_tile_pool(name="ps3c", bufs=1, space="PSUM")
                    OT_ps = ps3.tile([DH + 1, NPC], F32, tag="OT", bufs=1, name=f"OTps{h}")

                    def emit_attnv(kt, et):
                        for qh in range(2):
                            mi = nc.tensor.matmul(
                                out=OT_ps[:, qh * 512:(qh + 1) * 512],
                                lhsT=V4[:, kt, h, :],
                                rhs=et[:, qh * 512:(qh + 1) * 512],
                                start=(kt == 0), stop=(kt == NT_GLOB - 1))
                            if qh == 1:
                                mi.ins.ldweights = False  # same V tile as qh=0

                    # software-pipelined: attn@V runs two steps behind exp, so
                    # the PE consumes exp outputs produced ~2.5 us earlier and
                    # never waits on semaphore propagation from the ACT engine.
                    DEPTH = 2
                    et_hist = []
                    for kt in range(NT_GLOB):
                        if h >= 2 and kt % 3 == 2:
                            sps = ps3c[0].tile([P, NPC], F32, tag="sps2", bufs=1,
                                               name=f"sps2_{h}_{kt}")
                        else:
                            sps = ps3.tile([P, NPC], F32, tag="sps", bufs=2)
                        for qh in range(2):
                            nc.tensor.matmul(
                                out=sps[:, qh * 512:(qh + 1) * 512],
                                lhsT=KTp[p][po:po + DH, kt * P:(kt + 1) * P],
                                rhs=QTp[p][po:po + DH, qh * 512:(qh + 1) * 512],
                                start=True, stop=True)
                        et = ph3.tile([P, NPC], BF16, tag="expT", bufs=DEPTH + 2)
                        nc.scalar.activation(out=et[:], in_=sps[:], func=AF.Exp,
                                             bias=expb_col[:, 0:1], scale=1.0 / np.sqrt(DH))
                        et_hist.append(et)
                        if h < 2:
                            step = h * NT_GLOB + kt + 1
                            emit_scatter_jobs(n_jobs * step // n_steps)
                        if kt >= DEPTH:
                            emit_attnv(kt - DEPTH, et_hist[kt - DEPTH])
                    for kt in range(NT_GLOB - DEPTH, NT_GLOB):
                        emit_attnv(kt, et_hist[kt])

                    # drain PSUM (releases the accumulator); normalization by
                    # the denominator happens at the start of phase 4.
                    nc.vector.tensor_copy(out=OTu_sb[h][:], in_=OT_ps[:])
                if ps3c[0] is not None:
                    ps3c[0].release()

            # ================= phase 4: out_proj, LN, combine, fc =================
            with tc.tile_pool(name="ph4", bufs=1) as ph4:
                Wop_t = ph4.tile([DH, H * C], BF16)
                nc.sync.dma_start(out=Wop_t[:], in_=Wop4[:])
                Wl_t = ph4.tile([P, 2 * C], BF16)
                nc.sync.dma_start(out=Wl_t[:].rearrange("p (c n) -> p c n", c=2), in_=Wl[:].rearrange("(c p) n -> p c n", p=P))
                fc_t = ph4.tile([P, 2 * OUTC], BF16)
                nc.sync.dma_start(out=fc_t[:].rearrange("p (c n) -> p c n", c=2), in_=fcT[:].rearrange("(c p) n -> p c n", p=P))
                opb_t = ph4.tile([P, C], F32)
                nc.sync.dma_start(out=opb_t[:], in_=opb_rep[:])
                g_t = ph4.tile([P, C], F32)
                nc.sync.dma_start(out=g_t[:], in_=g_rep[:])
                b_t = ph4.tile([P, C], F32)
                nc.sync.dma_start(out=b_t[:], in_=b_rep[:])
                identf = ph4.tile([P, P], F32)
                nc.sync.dma_start(out=identf[:], in_=ident32_in[:])
                # fold (1-w) of the local/global mix into the LN gain and bias
                nc.vector.tensor_scalar(out=g_t[:], in0=g_t[:], scalar1=w1m_col[:, 0:1],
                                        scalar2=None, op0=OP.mult)
                nc.vector.tensor_scalar(out=b_t[:], in0=b_t[:], scalar1=w1m_col[:, 0:1],
                                        scalar2=None, op0=OP.mult)
                fcb_t = ph4.tile([P, OUTC], F32)
                nc.sync.dma_start(out=fcb_t[:], in_=fcb_rep[:])

                # ---- softmax denominators: transpose to node-major via tiny
                # ones-matmuls, single cheap reciprocal; 1/den is applied
                # per-query inside the qt loop below.
                with tc.tile_pool(name="ps4a", bufs=1, space="PSUM") as ps4a:
                    on64 = ph4.tile([P, 1], BF16)
                    nc.vector.memset(on64[:], 1.0)
                    # all 32 per-(qt,head) denominator columns land in one PSUM
                    # tile (separate single-column matmul groups) -> one drain
                    dps = ps4a.tile([P, H * NT_LOC], F32, tag="dps", bufs=1)
                    for qt in range(NT_LOC):
                        for h in range(H):
                            nc.tensor.matmul(
                                out=dps[:, qt * H + h:qt * H + h + 1],
                                lhsT=OTu_sb[h][DH:DH + 1, qt * P:(qt + 1) * P],
                                rhs=on64[64:65, 0:1],
                                start=True, stop=True)
                    den_cols = ph4.tile([P, H * NT_LOC], F32)
                    nc.vector.tensor_copy(out=den_cols[:], in_=dps[:])
                    rden_cols = ph4.tile([P, H * NT_LOC], F32)
                    nc.vector.reciprocal(out=rden_cols[:], in_=den_cols[:])

                ps4 = tc.alloc_tile_pool(name="ps4", bufs=1, space="PSUM")

                def transpose_2chunks(src_ap, tag, on_act=False):
                    # f32 transpose with converting bf16 drains, split ACT/DVE
                    dst = ph4.tile([P, C], BF16, tag=tag, bufs=2, name=f"t2c{tag}")
                    for c in range(2):
                        tp = ps4.tile([P, P], F32, tag="tp", bufs=2)
                        nc.tensor.transpose(out=tp[:], in_=src_ap[:, c * P:(c + 1) * P],
                                            identity=identf[:])
                        if on_act:
                            if c == 0:
                                nc.scalar.copy(out=dst[:, c * P:(c + 1) * P], in_=tp[:])
                            else:
                                nc.vector.tensor_copy(out=dst[:, c * P:(c + 1) * P], in_=tp[:])
                        else:
                            nc.vector.tensor_copy(out=dst[:, c * P:(c + 1) * P], in_=tp[:])
                    return dst

                for qt in range(NT_LOC):
                    # ---- global path: per-head out_proj, scaled by 1/den per
                    # query while summing heads (ACT/DVE split) ----
                    xo_t = ph4.tile([P, C], F32, tag="xot", bufs=2)
                    nc.sync.dma_start(out=xo_t[:], in_=x_own[qt * P:(qt + 1) * P, :])
                    cps = [ps4.tile([P, C], F32, tag="cps", bufs=4, name=f"cps{h}_{qt}")
                           for h in range(H)]
                    for h in range(H):
                        nc.tensor.matmul(
                            out=cps[h][:],
                            lhsT=OTu_sb[h][0:DH, qt * P:(qt + 1) * P],
                            rhs=Wop_t[:, h * C:(h + 1) * C],
                            start=True, stop=True)
                    vs = [ph4.tile([P, C], F32, tag="vs", bufs=8, name=f"vs{h}_{qt}")
                          for h in range(H)]
                    for h in range(H):
                        r_ap = rden_cols[:, qt * H + h:qt * H + h + 1]
                        if h % 2 == 0:
                            nc.scalar.activation(out=vs[h][:], in_=cps[h][:],
                                                 func=AF.Copy, scale=r_ap)
                        else:
                            nc.vector.tensor_scalar(out=vs[h][:], in0=cps[h][:],
                                                    scalar1=r_ap, scalar2=None,
                                                    op0=OP.mult)
                    xob = ph4.tile([P, C], F32, tag="xob", bufs=2)
                    nc.gpsimd.tensor_tensor(out=xob[:], in0=xo_t[:], in1=opb_t[:],
                                            op=OP.add)
                    nc.vector.tensor_tensor(out=vs[0][:], in0=vs[0][:], in1=vs[1][:],
                                            op=OP.add)
                    nc.gpsimd.tensor_tensor(out=vs[2][:], in0=vs[2][:], in1=vs[3][:],
                                            op=OP.add)
                    nc.vector.tensor_tensor(out=vs[0][:], in0=vs[0][:], in1=vs[2][:],
                                            op=OP.add)
                    v_t = ph4.tile([P, C], F32, tag="vt", bufs=2)
                    nc.vector.tensor_tensor(out=v_t[:], in0=vs[0][:], in1=xob[:],
                                            op=OP.add)
                    msum = ph4.tile([P, 1], F32, tag="msum", bufs=2)
                    nc.vector.reduce_sum(out=msum[:], in_=v_t[:], axis=AX.X)
                    mean_neg = ph4.tile([P, 1], F32, tag="mean", bufs=2)
                    nc.vector.tensor_scalar(out=mean_neg[:], in0=msum[:],
                                            scalar1=-1.0 / C, scalar2=None, op0=OP.mult)
                    nc.scalar.activation(out=v_t[:], in_=v_t[:], func=AF.Identity,
                                         bias=mean_neg[:, 0:1])
                    sq = ph4.tile([P, C], F32, tag="sq", bufs=2)
                    ssum = ph4.tile([P, 1], F32, tag="ssum", bufs=2)
                    nc.scalar.activation(out=sq[:], in_=v_t[:], func=AF.Square,
                                         accum_out=ssum[:])
                    sstd = ph4.tile([P, 1], F32, tag="sstd", bufs=2)
                    nc.scalar.activation(out=sstd[:], in_=ssum[:], func=AF.Sqrt,
                                         bias=eps_col[:, 0:1], scale=1.0 / C)
                    rstd = ph4.tile([P, 1], F32, tag="rstd", bufs=2)
                    nc.vector.reciprocal(out=rstd[:], in_=sstd[:])
                    nc.scalar.activation(out=v_t[:], in_=v_t[:], func=AF.Copy,
                                         scale=rstd[:, 0:1])
                    nc.vector.tensor_tensor(out=v_t[:], in0=v_t[:], in1=g_t[:], op=OP.mult)
                    nc.vector.tensor_tensor(out=v_t[:], in0=v_t[:], in1=b_t[:], op=OP.add)

                    # ---- local path: local_embed = hi @ W_local ----
                    hiT = transpose_2chunks(hi_sb[qt][:], "hiT", on_act=True)
                    lps = ps4.tile([P, C], F32, tag="lps", bufs=2)
                    for c in range(2):
                        nc.tensor.matmul(out=lps[:], lhsT=hiT[:, c * P:(c + 1) * P],
                                         rhs=Wl_t[:, c * C:(c + 1) * C],
                                         start=(c == 0), stop=(c == 1))
                    # combined = w*local + (1-w)*global; the (1-w) factor is
                    # already inside v_t via the scaled LN gain/bias.
                    comb = ph4.tile([P, C], F32, tag="comb", bufs=2)
                    nc.scalar.activation(out=comb[:], in_=lps[:], func=AF.Copy,
                                         scale=w_col[:, 0:1])
                    nc.vector.tensor_tensor(out=comb[:], in0=comb[:], in1=v_t[:], op=OP.add)

                    # ---- fc ----
                    cT = transpose_2chunks(comb[:], "cT", on_act=True)
                    fps = ps4.tile([P, OUTC], F32, tag="lps", bufs=2, name=f"fps{qt}")
                    for c in range(2):
                        nc.tensor.matmul(out=fps[:], lhsT=cT[:, c * P:(c + 1) * P],
                                         rhs=fc_t[:, c * OUTC:(c + 1) * OUTC],
                                         start=(c == 0), stop=(c == 1))
                    o_t = ph4.tile([P, OUTC], F32, tag="ot", bufs=2)
                    nc.vector.tensor_tensor(out=o_t[:], in0=fps[:], in1=fcb_t[:], op=OP.add)
                    nc.sync.dma_start(out=out[qt * P:(qt + 1) * P, :], in_=o_t[:])
                ps4.release()
    nc.finalize()
    return nc


def _prep_edges(adj):
    """Bucket edges by destination node-tile; pad segments to a common length.

    Pure relayout/counting on the host: per-core local col indices, per-edge
    source-row indices (used to gather x into edge order), integer degrees.
    """
    row = np.asarray(adj[0], dtype=np.int64)
    col = np.asarray(adj[1], dtype=np.int64)
    d = np.bincount(col, minlength=N).astype(np.int32)
    tid = col // P
    order = np.argsort(tid, kind='stable')
    row_s, col_s = row[order], col[order]
    counts = np.bincount(tid, minlength=NT_GLOB)
    S = int(np.ceil(max(counts.max(), 1) / P) * P)
    TPT = S // P
    col_pad = np.full((NT_GLOB, S), -1, dtype=np.int32)
    row_pad = np.zeros((NT_GLOB, S), dtype=np.int32)
    start = 0
    for g in range(NT_GLOB):
        cnt = int(counts[g])
        col_pad[g, :cnt] = (col_s[start:start + cnt] - g * P).astype(np.int32)
        row_pad[g, :cnt] = row_s[start:start + cnt].astype(np.int32)
        start += cnt
    TE = NT_LOC * TPT
    per_core = []
    for k in range(NCORES):
        cols_k = col_pad[NT_LOC * k:NT_LOC * (k + 1)].reshape(TE, P)
        rows_k = row_pad[NT_LOC * k:NT_LOC * (k + 1)].reshape(TE, P)
        ca = np.ascontiguousarray(cols_k.T)                    # [P, TE]
        dre = np.ascontiguousarray(d[rows_k].T)                # [P, TE]
        down = np.ascontiguousarray(
            d[k * NPC:(k + 1) * NPC].reshape(NT_LOC, P).T)     # [P, NT_LOC]
        per_core.append((ca, rows_k.reshape(-1), dre, down))
    return per_core, TPT


def kernel(x, adj, weight_local, in_proj_w, in_proj_b, out_proj_w, out_proj_b,
           ln_g, ln_b, alpha, fc_w, fc_b):
    global LAST_RESULTS
    x = np.ascontiguousarray(np.asarray(x, dtype=np.float32))
    per_core_edges, TPT = _prep_edges(np.asarray(adj))

    bf = ml_dtypes.bfloat16
    x_bf = x.astype(bf)
    xT_bf = np.ascontiguousarray(x_bf.T)
    WopT = np.asarray(out_proj_w).T.astype(np.float32)  # [C_in, C_out]
    Wop4 = np.ascontiguousarray(
        WopT.reshape(H, DH, C).transpose(1, 0, 2).reshape(DH, H * C).astype(bf))
    common = dict(
        xT=xT_bf,
        WqT=np.ascontiguousarray(np.asarray(in_proj_w)[0:C].T.astype(bf)),
        WkT=np.ascontiguousarray(np.asarray(in_proj_w)[C:2 * C].T.astype(bf)),
        WvT=np.ascontiguousarray(np.asarray(in_proj_w)[2 * C:3 * C].T.astype(bf)),
        Wop4=Wop4,
        Wl=np.ascontiguousarray(np.asarray(weight_local).astype(bf)),
        fcT=np.ascontiguousarray(np.asarray(fc_w).T.astype(bf)),
        bq_pack=np.ascontiguousarray(np.asarray(in_proj_b)[0:C].astype(np.float32).reshape(2, P).T),
        bk_pack=np.ascontiguousarray(np.asarray(in_proj_b)[C:2 * C].astype(np.float32).reshape(2, P).T),
        bv_rep=np.tile(np.asarray(in_proj_b)[2 * C:3 * C].astype(np.float32), (P, 1)),
        opb_rep=np.tile(np.asarray(out_proj_b, dtype=np.float32), (P, 1)),
        g_rep=np.tile(np.asarray(ln_g, dtype=np.float32), (P, 1)),
        b_rep=np.tile(np.asarray(ln_b, dtype=np.float32), (P, 1)),
        fcb_rep=np.tile(np.asarray(fc_b, dtype=np.float32), (P, 1)),
        alpha11=np.asarray(alpha, dtype=np.float32).reshape(1, 1),
        iota_in=np.tile(np.arange(P, dtype=np.float32), (P, 1)).astype(bf),
        ident32_in=np.eye(P, dtype=np.float32),
        ident_in=np.eye(P, dtype=np.float32).astype(bf),
        ones_row_in=np.ones((1, P), dtype=np.float32),
    )
    in_maps = []
    for k in range(NCORES):
        ca, rows_flat, dre, down = per_core_edges[k]
        m = dict(common)
        m['xT_own'] = np.ascontiguousarray(xT_bf[:, k * NPC:(k + 1) * NPC])
        m['x_own'] = np.ascontiguousarray(x[k * NPC:(k + 1) * NPC, :])
        m['xe'] = np.ascontiguousarray(x_bf[rows_flat])
        m['col_adj'] = ca
        m['dr_edge'] = dre
        m['d_own'] = down
        in_maps.append(m)

    nc = _build(TPT)
    res = run_bass_kernel_spmd(nc, in_maps, core_ids=list(range(NCORES)))
    LAST_RESULTS = res
    return np.concatenate([res.results[k]['out'] for k in range(NCORES)], axis=0)
